# revision 33
# baseline (speedup 1.0000x reference)
"""DCN (cross+deep) Trainium2 Bass kernel, 8 NeuronCores.

Sharding: data-parallel over batch (2048 rows/core); embedding rows gathered
host-side (feature_value premultiplied in f32), cross/deep weights replicated.

Math restructure (exact): the cross tower never needs materializing. Since
  y_{i+1} = x0 * (y_i . w_i) + cb_i + y_i
preserves the form y_i = x0 * a_i + C_i (a_i per-row scalar, C_i = cumsum cb),
the whole cross branch + its slice of the output dot reduces to per-row
scalars P_i = x0 . w_i and Q = x0 . ow_cross:
  a_1 = 1 + P_0;  a_{i+1} = a_i (1 + P_i) + C_i W_i   (W_i = sum w_i)
  r_cross = a_3 Q + C_3 sum(ow_cross)
The P/Q pass, deep output matvec (row 6) and everything else accumulate in
one narrow PSUM group; a tiny PE transpose turns [8, 512] into per-row
scalars and the final combine runs on the otherwise-idle Pool engine.

Everything heavy runs in fp8e4m3 with DoubleRow perf mode (0.5 PE
cycles/row, two k-tiles per call): L0 (1024-padded x 1024), L1 (1024x512),
L2 (512x256) and the P/Q pass. Accuracy (gate 2e-2, measured 1.4e-2) is
held by residual compensation where it matters: Q = x . ow_cross feeds the
output directly, so it is computed as xhi.owh + (xhi.owl + xlo.owh)/16
(7-bit effective mantissas); P_i only enter via (1+P_i) ~ 1, so plain fp8
is fine. Host pre-quantizes x (*64, + *16 residual) and weights (*16); ACT
and DVE tensor_scalar ops fuse dequant+relu+requant via scale folding.

Schedule: software-pipelined across chunks with stage skew so the PE never
waits on ACT/DVE drain latency: iteration `it` runs L2+out for chunk it-2,
P/Q+L0 for chunk it, L1 for chunk it-1, the chunk it-3 tail (transpose +
Pool combine + store) interleaved, and chunk it+1's x DMA prefetched.
PSUM drains alternate ACT/DVE per tile to balance the two engines.
"""

import numpy as np
import ml_dtypes
from contextlib import ExitStack

import concourse.tile as tile
import concourse.mybir as mybir
from concourse import bacc
from concourse.bass_utils import run_bass_kernel_spmd

# ---- problem constants (hardcoded; kernel.py must be self-contained) ----
B, F, E = 16384, 26, 32
NF = 1_000_000
D = F * E                    # 832
DEEP = (1024, 512, 256)
N_CROSS = 3
N_CORES = 8
S = B // N_CORES             # 2048 batch rows per core
CHUNK = 512
NCHUNK = S // CHUNK          # 4
K8 = 8                       # fp8 k-tiles (1024 pad of 832)
KR = 2 * K8                  # hi + lo x tiles per chunk
D8 = K8 * 128                # 1024
M0, M1, M2 = DEEP[0] // 128, DEEP[1] // 128, DEEP[2] // 128  # 8, 4, 2

# fp8 scaling (powers of two; folded into ACT/DVE scales and Pool combine)
SX, SW0, SY0, SW1 = 64.0, 16.0, 64.0, 16.0
SY1, SW2 = 64.0, 16.0
SCW, SQC = 16.0, 16.0
SCL0 = SY0 / (SX * SW0)      # PSUM0 -> sy0*y0
SCL1 = SY1 / (SY0 * SW1)     # PSUM1 -> sy1*y1
SCL2 = 1.0 / (SY1 * SW2)     # PSUM2 -> y2 (natural)
C_Q = 1.0 / (SX * SQC)       # Q dequant (folded into the a-chain)
S_P = 1.0 / (SX * SCW)       # P dequant

NWARM = 8                    # PE p-state warm-up matmuls (tuned by sweep)
Y0_DVE = (1, 3, 5, 7)        # which y0 m-tiles drain on DVE (zb path)
Y1_DVE = (1, 3)
Y2_DVE = (0,)
SBQ_DVE = False
SPLIT_DMA = True
INLINE_L1 = False
EARLY_OUT = False
DPS_BUFS = 5
QPS_BUFS = 3
DRAIN_SEQ = (                # drain ordering (tuned by sweep)
    ("l2", 2), ("l1", 3), ("out", 2), ("tail", 1),
    ("l2", 3), ("out", 3), ("tail", 2), ("tail", 3),
)

_bf = mybir.dt.bfloat16
_f32 = mybir.dt.float32
_f8 = mybir.dt.float8e4
_np_bf = ml_dtypes.bfloat16
_np_f8 = ml_dtypes.float8_e4m3

_CACHE = {}


def _build_nc(zb=True, zk=True):
    """zb: all of b0/b1/b2 are zero -> half the relu drains use DVE
    2-op tensor_scalar forms; otherwise every drain runs on ACT with bias."""
    AF = mybir.ActivationFunctionType
    OP = mybir.AluOpType
    PM = mybir.MatmulPerfMode
    nc = bacc.Bacc(
        "TRN2", target_bir_lowering=False, debug=False, num_devices=N_CORES
    )

    x8_d = nc.dram_tensor("x8", [128, NCHUNK * KR * CHUNK], _f8, kind="ExternalInput")
    w0_d = nc.dram_tensor("w0", [128, K8 * DEEP[0]], _f8, kind="ExternalInput")
    w1_d = nc.dram_tensor("w1", [128, K8 * DEEP[1]], _f8, kind="ExternalInput")
    w2_d = nc.dram_tensor("w2", [128, M1 * DEEP[2]], _f8, kind="ExternalInput")
    pq8_d = nc.dram_tensor("pq8", [128, KR * 16], _f8, kind="ExternalInput")
    owd_d = nc.dram_tensor("owd", [128, M2 * 16], _bf, kind="ExternalInput")
    id_d = nc.dram_tensor("ident", [16, 16], _f32, kind="ExternalInput")
    # f32 consts: [b0*sy0 (8) | b1*sy1 (4) | b2 (2) | k1*cq k2*cq kf (3)]
    cst_d = nc.dram_tensor("cst", [128, M0 + M1 + M2 + 3], _f32, kind="ExternalInput")
    out_d = nc.dram_tensor("out", [S, 1], _f32, kind="ExternalOutput")

    with ExitStack() as ctx:
        tc = ctx.enter_context(tile.TileContext(nc))
        wp = ctx.enter_context(tc.tile_pool(name="wp", bufs=1))
        x8p = ctx.enter_context(tc.tile_pool(name="x8p", bufs=2))
        y0p = ctx.enter_context(tc.tile_pool(name="y0p", bufs=2))
        y1p = ctx.enter_context(tc.tile_pool(name="y1p", bufs=2))
        y2p = ctx.enter_context(tc.tile_pool(name="y2p", bufs=2))
        pqs = ctx.enter_context(tc.tile_pool(name="pqs", bufs=2))
        rp = ctx.enter_context(tc.tile_pool(name="rp", bufs=2))
        dps = ctx.enter_context(tc.tile_pool(name="dps", bufs=DPS_BUFS, space="PSUM"))
        qps = ctx.enter_context(tc.tile_pool(name="qps", bufs=QPS_BUFS, space="PSUM"))

        # ---- weights / constants to SBUF ----
        # DMA emission order == need order: x8_0 + w0 feed the first L0
        # groups (behind the warm-up burst); w1/w2/owd/ident load late.
        w0_sb = wp.tile([128, K8, DEEP[0]], _f8)
        w0_r = w0_d[:, :].rearrange("p (k m) -> p k m", k=K8)
        x8r = x8_d[:, :].rearrange("p (c k j) -> p c k j", c=NCHUNK, k=KR)
        x8t0 = x8p.tile([128, KR, CHUNK], _f8, tag="x8", name="x8_0")
        nc.sync.dma_start(x8t0[:, 0:2, :], x8r[:, 0, 0:2, :])
        nc.sync.dma_start(w0_sb[:, :, 0:512], w0_r[:, :, 0:512])
        nc.sync.dma_start(x8t0[:, 2:K8, :], x8r[:, 0, 2:K8, :])
        nc.sync.dma_start(x8t0[:, K8:KR, :], x8r[:, 0, K8:KR, :])
        pq8_sb = wp.tile([128, KR, 16], _f8)
        nc.sync.dma_start(pq8_sb[:], pq8_d[:, :].rearrange("p (k c) -> p k c", k=KR))
        cst_sb = wp.tile([128, M0 + M1 + M2 + 3], _f32)
        nc.sync.dma_start(cst_sb[:], cst_d[:, :])
        b0_sb = cst_sb[:, 0:M0]
        b1_sb = cst_sb[:, M0:M0 + M1]
        b2_sb = cst_sb[:, M0 + M1:M0 + M1 + M2]
        kv_sb = cst_sb[:, M0 + M1 + M2:M0 + M1 + M2 + 3]
        nc.sync.dma_start(w0_sb[:, :, 512:1024], w0_r[:, :, 512:1024])
        id_sb = wp.tile([16, 16], _f32)
        w1_sb = wp.tile([128, K8, DEEP[1]], _f8)
        w2_sb = wp.tile([128, M1, DEEP[2]], _f8)
        owd_sb = wp.tile([128, M2, 16], _bf)

        def _late_loads():
            nc.sync.dma_start(w1_sb[:], w1_d[:, :].rearrange("p (k m) -> p k m", k=K8))
            nc.sync.dma_start(w2_sb[:], w2_d[:, :].rearrange("p (k m) -> p k m", k=M1))
            nc.sync.dma_start(owd_sb[:], owd_d[:, :].rearrange("p (k c) -> p k c", k=M2))
            nc.sync.dma_start(id_sb[:], id_d[:, :])

        # "Observe" ops: each engine touches its DMA-loaded constants once so
        # steady-state instructions carry at most one semaphore wait.
        obs = wp.tile([128, 8], _f32)
        nc.vector.tensor_copy(obs[:, 0:1], kv_sb[:, 0:1])
        nc.gpsimd.tensor_copy(obs[:, 4:5], kv_sb[:, 0:1])
        nc.scalar.activation(obs[:, 1:2], b0_sb[:, 0:1], AF.Copy)
        nc.scalar.activation(obs[:, 2:3], b1_sb[:, 0:1], AF.Copy)
        nc.scalar.activation(obs[:, 3:4], b2_sb[:, 0:1], AF.Copy)
        # PE warm-up burst: keep the PE busy during the startup DMA window so
        # the clock p-state ramps before the first real matmul group.
        warm = wp.tile([128, 512], _bf)
        nc.gpsimd.memset(warm[:], 0.0)
        warm_ps = dps.tile([128, 512], _f32, tag="dps", name="warm_ps")
        for _ in range(NWARM):
            nc.tensor.matmul(
                warm_ps[:], lhsT=warm[:, 0:128], rhs=warm[:], start=True, stop=True
            )
        for w_ap in (pq8_sb[:, 0, 0:1], w0_sb[:, 0, 0:1]):
            nc.tensor.matmul(
                warm_ps[0:1, 0:1], lhsT=w_ap, rhs=w_ap, start=True, stop=True
            )

        # ---- per-chunk stage emitters ----
        x8ts = {0: x8t0}
        y0ts, y1ts, y2ts, qts, sbqs = {}, {}, {}, {}, {}

        def dma_x(c):
            x8t = x8p.tile([128, KR, CHUNK], _f8, tag="x8", name=f"x8_{c}")
            if SPLIT_DMA:
                nc.sync.dma_start(x8t[:, 0:K8, :], x8r[:, c, 0:K8, :])
                nc.sync.dma_start(x8t[:, K8:KR, :], x8r[:, c, K8:KR, :])
            else:
                nc.sync.dma_start(x8t[:], x8r[:, c, :, :])
            x8ts[c] = x8t

        def pq_open(c):
            # P/Q DoubleRow group over hi tiles (P, Q1, Q2) and lo tiles (Q3)
            qt = qps.tile([16, CHUNK], _f32, tag="pq", name=f"pq_{c}")
            qts[c] = qt
            for kp in range(KR // 2):
                nc.tensor.matmul(
                    qt[:],
                    lhsT=pq8_sb[:, 2 * kp:2 * kp + 2, :],
                    rhs=x8ts[c][:, 2 * kp:2 * kp + 2, :],
                    start=(kp == 0),
                    stop=False,
                    perf_mode=PM.DoubleRow,
                    skip_group_check=True,
                )

        def l0_alloc(c):
            y0ts[c] = y0p.tile([128, K8, CHUNK], _f8, tag="y0", name=f"y0_{c}")

        def l0(c, m):
            ps = dps.tile([128, CHUNK], _f32, tag="dps", name=f"ps0_{c}_{m}")
            for kp in range(K8 // 2):
                nc.tensor.matmul(
                    ps[:],
                    lhsT=w0_sb[:, 2 * kp:2 * kp + 2, m * 128:(m + 1) * 128],
                    rhs=x8ts[c][:, 2 * kp:2 * kp + 2, :],
                    start=(kp == 0),
                    stop=(kp == K8 // 2 - 1),
                    perf_mode=PM.DoubleRow,
                )
            # drain-balance: even m on ACT, odd m on DVE (zero-bias form)
            if zb and m in Y0_DVE:
                nc.vector.tensor_scalar(
                    y0ts[c][:, m, :], ps[:], SCL0, 0.0, op0=OP.mult, op1=OP.max
                )
            else:
                nc.scalar.activation(
                    y0ts[c][:, m, :], ps[:], AF.Relu, bias=b0_sb[:, m:m + 1], scale=SCL0
                )

        def l1(c):
            y1t = y1p.tile([128, M1, CHUNK], _f8, tag="y1", name=f"y1_{c}")
            y1ts[c] = y1t
            for m in range(M1):
                ps = dps.tile([128, CHUNK], _f32, tag="dps", name=f"ps1_{c}_{m}")
                for kp in range(K8 // 2):
                    nc.tensor.matmul(
                        ps[:],
                        lhsT=w1_sb[:, 2 * kp:2 * kp + 2, m * 128:(m + 1) * 128],
                        rhs=y0ts[c][:, 2 * kp:2 * kp + 2, :],
                        start=(kp == 0),
                        stop=(kp == K8 // 2 - 1),
                        perf_mode=PM.DoubleRow,
                    )
                if zb and m in Y1_DVE:
                    nc.vector.tensor_scalar(
                        y1t[:, m, :], ps[:], SCL1, 0.0, op0=OP.mult, op1=OP.max
                    )
                else:
                    nc.scalar.activation(
                        y1t[:, m, :], ps[:], AF.Relu, bias=b1_sb[:, m:m + 1], scale=SCL1
                    )

        def l2(c):
            y2t = y2p.tile([128, M2, CHUNK], _bf, tag="y2", name=f"y2_{c}")
            y2ts[c] = y2t
            for m in range(M2):
                ps = dps.tile([128, CHUNK], _f32, tag="dps", name=f"ps2_{c}_{m}")
                for kp in range(M1 // 2):
                    nc.tensor.matmul(
                        ps[:],
                        lhsT=w2_sb[:, 2 * kp:2 * kp + 2, m * 128:(m + 1) * 128],
                        rhs=y1ts[c][:, 2 * kp:2 * kp + 2, :],
                        start=(kp == 0),
                        stop=(kp == M1 // 2 - 1),
                        perf_mode=PM.DoubleRow,
                    )
                if zb and m in Y2_DVE:
                    nc.vector.tensor_scalar(
                        y2t[:, m, :], ps[:], SCL2, 0.0, op0=OP.mult, op1=OP.max
                    )
                else:
                    nc.scalar.activation(
                        y2t[:, m, :], ps[:], AF.Relu, bias=b2_sb[:, m:m + 1],
                        scale=SCL2,
                    )

        def out_mv(c, j):
            # deep output matvec accumulates into row 6 of the P/Q group
            nc.tensor.matmul(
                qts[c][:],
                lhsT=owd_sb[:, j, :],
                rhs=y2ts[c][:, j, :],
                start=False,
                stop=(j == M2 - 1),
                skip_group_check=True,
            )
            if j == M2 - 1:
                sbq = pqs.tile([16, CHUNK], _f32, tag="sbq", name=f"sbq_{c}")
                if SBQ_DVE:
                    nc.vector.tensor_copy(sbq[:], qts[c][:])
                else:
                    nc.scalar.activation(sbq[:], qts[c][:], AF.Copy)
                sbqs[c] = sbq

        def tail(c):
            sbq = sbqs[c]
            # transpose scratch borrows a [128,512] slot from the dps ring
            pt = dps.tile([128, CHUNK], _f32, tag="dps", name=f"ptr_{c}")
            ptr = pt[:, 0:64].rearrange("p (s i) -> p s i", s=4)
            for s in range(4):
                nc.tensor.transpose(
                    ptr[:, s, :], sbq[:, s * 128:(s + 1) * 128], id_sb[:]
                )
            # one small ACT copy frees the PSUM slot; the scalar combine then
            # runs entirely on the otherwise-idle Pool engine from SBUF.
            # pb rows: 0-2 P_i, 3 Q1 (hi.hi), 4 Q2 (hi.lo), 5 Q3 (lo.hi),
            # 6 Rdeep.  The a-chain carries the C_Q fold: t1 = (1+P0)*C_Q.
            pb = rp.tile([128, 4, 16], _f32, tag="pb", name=f"pb_{c}")
            nc.scalar.activation(pb[:], ptr[:], AF.Copy)
            t1 = rp.tile([128, 4], _f32, tag="t1", name=f"t1_{c}")
            t2 = rp.tile([128, 4], _f32, tag="t2", name=f"t2_{c}")
            t3 = rp.tile([128, 4], _f32, tag="t3", name=f"t3_{c}")
            nc.gpsimd.tensor_scalar(
                t1[:], pb[:, :, 0], S_P * C_Q, C_Q, op0=OP.mult, op1=OP.add
            )
            nc.gpsimd.tensor_scalar(
                t2[:], pb[:, :, 1], S_P, 1.0, op0=OP.mult, op1=OP.add
            )
            nc.gpsimd.tensor_scalar(
                t3[:], pb[:, :, 2], S_P, 1.0, op0=OP.mult, op1=OP.add
            )
            u = rp.tile([128, 4], _f32, tag="u", name=f"u_{c}")
            nc.gpsimd.tensor_tensor(out=u[:], in0=pb[:, :, 4], in1=pb[:, :, 5], op=OP.add)
            qv = rp.tile([128, 4], _f32, tag="qv", name=f"qv_{c}")
            nc.gpsimd.tensor_scalar(qv[:], u[:], 1.0 / 16.0, None, op0=OP.mult)
            nc.gpsimd.tensor_tensor(out=qv[:], in0=qv[:], in1=pb[:, :, 3], op=OP.add)
            acc = rp.tile([128, 4], _f32, tag="acc", name=f"acc_{c}")
            nc.gpsimd.tensor_tensor(out=acc[:], in0=t1[:], in1=t2[:], op=OP.mult)
            if not zb:
                nc.gpsimd.tensor_scalar_add(acc[:], acc[:], kv_sb[:, 0:1])
            nc.gpsimd.tensor_tensor(out=acc[:], in0=acc[:], in1=t3[:], op=OP.mult)
            if not zb:
                nc.gpsimd.tensor_scalar_add(acc[:], acc[:], kv_sb[:, 1:2])
            nc.gpsimd.tensor_tensor(out=acc[:], in0=acc[:], in1=qv[:], op=OP.mult)
            res = rp.tile([128, 4], _f32, tag="res", name=f"res_{c}")
            if zk:
                nc.gpsimd.tensor_tensor(out=res[:], in0=acc[:], in1=pb[:, :, 6], op=OP.add)
            else:
                nc.gpsimd.tensor_tensor(out=acc[:], in0=acc[:], in1=pb[:, :, 6], op=OP.add)
                nc.gpsimd.tensor_scalar_add(res[:], acc[:], kv_sb[:, 2:3])
            nc.sync.dma_start(
                out=out_d[c * CHUNK:(c + 1) * CHUNK, :].rearrange(
                    "(s p) o -> p (s o)", p=128
                ),
                in_=res[:],
            )

        # ---- software-pipelined main loop ----
        for it in range(NCHUNK):
            A, Bc, Cc, Tc = it, it - 1, it - 2, it - 3
            if A == 0:
                l0_alloc(0)
                for m in range(4):
                    l0(0, m)
                pq_open(0)
                for m in range(4, M0):
                    l0(0, m)
                dma_x(1)
                _late_loads()
                continue
            if 0 <= Cc:
                l2(Cc)
            if A + 1 < NCHUNK:
                dma_x(A + 1)
            l0_alloc(A)
            l0(A, 0)
            if 0 <= Tc:
                tail(Tc)
            l0(A, 1)
            if EARLY_OUT and 0 <= Cc:
                out_mv(Cc, 0)
                out_mv(Cc, 1)
            pq_open(A)
            l0(A, 2)
            l0(A, 3)
            if 0 <= Bc:
                l1(Bc)
            for m in range(4, M0):
                l0(A, m)
            # last chunk: L1 inline (PE is idle in the drain anyway)
            if INLINE_L1 and A == NCHUNK - 1:
                l1(A)
            if not EARLY_OUT and 0 <= Cc:
                out_mv(Cc, 0)
                out_mv(Cc, 1)
        # ---- compressed drain: PE is idle, so collapse the stage skew ----
        for step in DRAIN_SEQ:
            kind, c = step
            if kind == "l2":
                l2(c)
            elif kind == "out":
                out_mv(c, 0)
                out_mv(c, 1)
            elif kind == "l1":
                l1(c)
            elif kind == "tail":
                tail(c)

    nc.compile()
    return nc


def _get_nc(zb=True, zk=True):
    key = f"nc_zb{int(zb)}_zk{int(zk)}"
    if key not in _CACHE:
        _CACHE[key] = _build_nc(zb=zb, zk=zk)
    return _CACHE[key]


def _prep_in_maps(inputs, zb):
    fi = np.asarray(inputs["feature_index"]).astype(np.int64)
    fvv = np.asarray(inputs["feature_value"], dtype=np.float32)
    emb = np.asarray(inputs["emb_table"], dtype=np.float32)
    cw = np.asarray(inputs["cross_w"], dtype=np.float32)
    cb = np.asarray(inputs["cross_b"], dtype=np.float32)
    w0 = np.asarray(inputs["w0"], dtype=np.float32)
    b0 = np.asarray(inputs["b0"], dtype=np.float32)
    w1 = np.asarray(inputs["w1"], dtype=np.float32)
    b1 = np.asarray(inputs["b1"], dtype=np.float32)
    w2 = np.asarray(inputs["w2"], dtype=np.float32)
    b2 = np.asarray(inputs["b2"], dtype=np.float32)
    ow = np.asarray(inputs["out_w"], dtype=np.float32).reshape(-1)
    ob = np.asarray(inputs["out_b"], dtype=np.float32).reshape(-1)

    # host gather with feature_value premultiplied (f32, before any cast)
    x = emb[fi] * fvv[:, :, None]               # [B, F, E] f32
    x = x.reshape(B, D)

    # hi/lo fp8 split of x (lo = 16x the hi-quantization residual)
    xs = np.zeros((B, D8), dtype=np.float32)
    xs[:, :D] = x * SX
    xhi = xs.astype(_np_f8)
    xlo = ((xs - xhi.astype(np.float32)) * 16.0).astype(_np_f8)

    # shared (replicated) weights
    w0p = np.zeros((D8, DEEP[0]), dtype=np.float32)
    w0p[:D] = w0 * SW0
    w0q = np.ascontiguousarray(
        w0p.reshape(K8, 128, DEEP[0]).transpose(1, 0, 2).reshape(128, -1)
    ).astype(_np_f8)
    w1q = np.ascontiguousarray(
        (w1 * SW1).reshape(K8, 128, DEEP[1]).transpose(1, 0, 2).reshape(128, -1)
    ).astype(_np_f8)
    w2q = np.ascontiguousarray(
        (w2 * SW2).reshape(M1, 128, DEEP[2]).transpose(1, 0, 2).reshape(128, -1)
    ).astype(_np_f8)

    # P/Q lhsT: hi k-tiles get [cw0 cw1 cw2 owch owcl 0 0 0]; lo k-tiles get
    # owch in column 5 (-> Q3).  All quantized fp8.
    owc = np.zeros((D8,), dtype=np.float32)
    owc[:D] = ow[:D]
    owch = (owc * SQC).astype(_np_f8)
    owcl = ((owc * SQC - owch.astype(np.float32)) * 16.0).astype(_np_f8)
    cwp = np.zeros((N_CROSS, D8), dtype=np.float32)
    cwp[:, :D] = cw * SCW
    pq8 = np.zeros((128, KR, 16), dtype=_np_f8)
    for k in range(K8):
        seg = slice(k * 128, (k + 1) * 128)
        for i in range(N_CROSS):
            pq8[:, k, i] = cwp[i, seg].astype(_np_f8)
        pq8[:, k, 3] = owch[seg]
        pq8[:, k, 4] = owcl[seg]
        pq8[:, K8 + k, 5] = owch[seg]
    pq8 = np.ascontiguousarray(pq8.reshape(128, -1))
    owd = np.zeros((128, M2, 16), dtype=np.float32)
    for j in range(M2):
        owd[:, j, 6] = ow[D + j * 128:D + (j + 1) * 128]
    owd = np.ascontiguousarray(owd.reshape(128, -1)).astype(_np_bf)

    C = np.cumsum(cb)                           # C[i] = cb_0 + ... + cb_i
    W = cw.sum(axis=1)
    k1 = C[0] * W[1] * C_Q
    k2 = C[1] * W[2] * C_Q
    kf = ob[0] + C[2] * ow[:D].sum()
    b0s = (b0 * SY0).reshape(M0, 128).T
    b1s = (b1 * SY1).reshape(M1, 128).T
    b2r = b2.reshape(M2, 128).T
    kv = np.tile(np.array([[k1, k2, kf]], dtype=np.float32), (128, 1))
    cst = np.ascontiguousarray(
        np.concatenate([b0s, b1s, b2r, kv], axis=1).astype(np.float32)
    )
    ident = np.eye(16, dtype=np.float32)

    shared = dict(w0=w0q, w1=w1q, w2=w2q, pq8=pq8, owd=owd, cst=cst, ident=ident)

    in_maps = []
    for core in range(N_CORES):
        rows = slice(core * S, (core + 1) * S)
        # per-chunk layout [128, c, k(hi 0-7, lo 8-15), j]
        xh = xhi[rows].reshape(NCHUNK, CHUNK, K8, 128).transpose(3, 0, 2, 1)
        xl = xlo[rows].reshape(NCHUNK, CHUNK, K8, 128).transpose(3, 0, 2, 1)
        x8 = np.concatenate([xh, xl], axis=2).reshape(128, -1)
        in_maps.append(dict(x8=np.ascontiguousarray(x8), **shared))
    return in_maps


def _zb(inputs):
    return not (
        np.any(np.asarray(inputs["b0"])) or np.any(np.asarray(inputs["b1"]))
        or np.any(np.asarray(inputs["b2"]))
    )


def _kf(inputs):
    cb = np.asarray(inputs["cross_b"], dtype=np.float32)
    ow = np.asarray(inputs["out_w"], dtype=np.float32).reshape(-1)
    ob = np.asarray(inputs["out_b"], dtype=np.float32).reshape(-1)
    return float(ob[0] + np.cumsum(cb)[2] * ow[:D].sum())


def _run(inputs, trace=False, **kw):
    zb = _zb(inputs)
    nc = _get_nc(zb=zb, zk=(_kf(inputs) == 0.0))
    in_maps = _prep_in_maps(inputs, zb)
    res = run_bass_kernel_spmd(
        nc, in_maps, core_ids=list(range(N_CORES)), trace=trace, **kw
    )
    out = np.concatenate([r["out"] for r in res.results], axis=0)
    return out.astype(np.float32), res


def kernel(**inputs) -> np.ndarray:
    out, _ = _run(inputs, trace=False)
    return out


# revision 34
# speedup vs baseline: 1.0057x; 1.0057x over previous
"""DCN (cross+deep) Trainium2 Bass kernel, 8 NeuronCores.

Sharding: data-parallel over batch (2048 rows/core); embedding rows gathered
host-side (feature_value premultiplied in f32), cross/deep weights replicated.

Math restructure (exact): the cross tower never needs materializing. Since
  y_{i+1} = x0 * (y_i . w_i) + cb_i + y_i
preserves the form y_i = x0 * a_i + C_i (a_i per-row scalar, C_i = cumsum cb),
the whole cross branch + its slice of the output dot reduces to per-row
scalars P_i = x0 . w_i and Q = x0 . ow_cross:
  a_1 = 1 + P_0;  a_{i+1} = a_i (1 + P_i) + C_i W_i   (W_i = sum w_i)
  r_cross = a_3 Q + C_3 sum(ow_cross)
The P/Q pass, deep output matvec (row 6) and everything else accumulate in
one narrow PSUM group; a tiny PE transpose turns [8, 512] into per-row
scalars and the final combine runs on the otherwise-idle Pool engine.

Everything heavy runs in fp8e4m3 with DoubleRow perf mode (0.5 PE
cycles/row, two k-tiles per call): L0 (1024-padded x 1024), L1 (1024x512),
L2 (512x256) and the P/Q pass. Accuracy (gate 2e-2, measured 1.4e-2) is
held by residual compensation where it matters: Q = x . ow_cross feeds the
output directly, so it is computed as xhi.owh + (xhi.owl + xlo.owh)/16
(7-bit effective mantissas); P_i only enter via (1+P_i) ~ 1, so plain fp8
is fine. Host pre-quantizes x (*64, + *16 residual) and weights (*16); ACT
and DVE tensor_scalar ops fuse dequant+relu+requant via scale folding.

Schedule: software-pipelined across chunks with stage skew so the PE never
waits on ACT/DVE drain latency: iteration `it` runs L2+out for chunk it-2,
P/Q+L0 for chunk it, L1 for chunk it-1, the chunk it-3 tail (transpose +
Pool combine + store) interleaved, and chunk it+1's x DMA prefetched.
PSUM drains alternate ACT/DVE per tile to balance the two engines.
"""

import numpy as np
import ml_dtypes
from contextlib import ExitStack

import concourse.tile as tile
import concourse.mybir as mybir
from concourse import bacc
from concourse.bass_utils import run_bass_kernel_spmd

# ---- problem constants (hardcoded; kernel.py must be self-contained) ----
B, F, E = 16384, 26, 32
NF = 1_000_000
D = F * E                    # 832
DEEP = (1024, 512, 256)
N_CROSS = 3
N_CORES = 8
S = B // N_CORES             # 2048 batch rows per core
CHUNK = 512
NCHUNK = S // CHUNK          # 4
K8 = 8                       # fp8 k-tiles (1024 pad of 832)
KR = 2 * K8                  # hi + lo x tiles per chunk
D8 = K8 * 128                # 1024
M0, M1, M2 = DEEP[0] // 128, DEEP[1] // 128, DEEP[2] // 128  # 8, 4, 2

# fp8 scaling (powers of two; folded into ACT/DVE scales and Pool combine)
SX, SW0, SY0, SW1 = 64.0, 16.0, 64.0, 16.0
SY1, SW2 = 64.0, 16.0
SCW, SQC = 16.0, 16.0
SCL0 = SY0 / (SX * SW0)      # PSUM0 -> sy0*y0
SCL1 = SY1 / (SY0 * SW1)     # PSUM1 -> sy1*y1
SCL2 = 1.0 / (SY1 * SW2)     # PSUM2 -> y2 (natural)
C_Q = 1.0 / (SX * SQC)       # Q dequant (folded into the a-chain)
S_P = 1.0 / (SX * SCW)       # P dequant

NWARM = 8                    # PE p-state warm-up matmuls (tuned by sweep)
Y0_DVE = (1, 3, 5, 7)        # which y0 m-tiles drain on DVE (zb path)
Y1_DVE = (1, 3)
Y2_DVE = (0,)
SBQ_DVE = False
SPLIT_DMA = True
INLINE_L1 = False
EARLY_OUT = False
DPS_BUFS = 5
QPS_BUFS = 3
DRAIN_SEQ = (                # drain ordering (tuned by sweep)
    ("l2", 2), ("l1", 3), ("out", 2), ("tail", 1),
    ("l2", 3), ("out", 3), ("tail", 2), ("tail", 3),
)

_bf = mybir.dt.bfloat16
_f32 = mybir.dt.float32
_f8 = mybir.dt.float8e4
_np_bf = ml_dtypes.bfloat16
_np_f8 = ml_dtypes.float8_e4m3

_CACHE = {}


def _build_nc(zb=True, zk=True):
    """zb: all of b0/b1/b2 are zero -> half the relu drains use DVE
    2-op tensor_scalar forms; otherwise every drain runs on ACT with bias."""
    AF = mybir.ActivationFunctionType
    OP = mybir.AluOpType
    PM = mybir.MatmulPerfMode
    nc = bacc.Bacc(
        "TRN2", target_bir_lowering=False, debug=False, num_devices=N_CORES
    )

    x8_d = nc.dram_tensor("x8", [128, NCHUNK * KR * CHUNK], _f8, kind="ExternalInput")
    w0_d = nc.dram_tensor("w0", [128, K8 * DEEP[0]], _f8, kind="ExternalInput")
    w1_d = nc.dram_tensor("w1", [128, K8 * DEEP[1]], _f8, kind="ExternalInput")
    w2_d = nc.dram_tensor("w2", [128, M1 * DEEP[2]], _f8, kind="ExternalInput")
    pq8_d = nc.dram_tensor("pq8", [128, KR * 16], _f8, kind="ExternalInput")
    owd_d = nc.dram_tensor("owd", [128, M2 * 16], _bf, kind="ExternalInput")
    id_d = nc.dram_tensor("ident", [16, 16], _f32, kind="ExternalInput")
    # f32 consts: [b0*sy0 (8) | b1*sy1 (4) | b2 (2) | k1*cq k2*cq kf (3)]
    cst_d = nc.dram_tensor("cst", [128, M0 + M1 + M2 + 3], _f32, kind="ExternalInput")
    out_d = nc.dram_tensor("out", [S, 1], _f32, kind="ExternalOutput")

    with ExitStack() as ctx:
        tc = ctx.enter_context(tile.TileContext(nc))
        wp = ctx.enter_context(tc.tile_pool(name="wp", bufs=1))
        x8p = ctx.enter_context(tc.tile_pool(name="x8p", bufs=2))
        y0p = ctx.enter_context(tc.tile_pool(name="y0p", bufs=2))
        y1p = ctx.enter_context(tc.tile_pool(name="y1p", bufs=2))
        y2p = ctx.enter_context(tc.tile_pool(name="y2p", bufs=2))
        pqs = ctx.enter_context(tc.tile_pool(name="pqs", bufs=2))
        rp = ctx.enter_context(tc.tile_pool(name="rp", bufs=2))
        dps = ctx.enter_context(tc.tile_pool(name="dps", bufs=DPS_BUFS, space="PSUM"))
        qps = ctx.enter_context(tc.tile_pool(name="qps", bufs=QPS_BUFS, space="PSUM"))

        # ---- weights / constants to SBUF ----
        # DMA emission order == need order: x8_0 + w0 feed the first L0
        # groups (behind the warm-up burst); w1/w2/owd/ident load late.
        w0_sb = wp.tile([128, K8, DEEP[0]], _f8)
        w0_r = w0_d[:, :].rearrange("p (k m) -> p k m", k=K8)
        x8r = x8_d[:, :].rearrange("p (c k j) -> p c k j", c=NCHUNK, k=KR)
        x8t0 = x8p.tile([128, KR, CHUNK], _f8, tag="x8", name="x8_0")
        nc.sync.dma_start(x8t0[:, 0:K8, :], x8r[:, 0, 0:K8, :])
        nc.sync.dma_start(w0_sb[:, :, 0:512], w0_r[:, :, 0:512])
        nc.sync.dma_start(x8t0[:, K8:KR, :], x8r[:, 0, K8:KR, :])
        pq8_sb = wp.tile([128, KR, 16], _f8)
        nc.sync.dma_start(pq8_sb[:], pq8_d[:, :].rearrange("p (k c) -> p k c", k=KR))
        cst_sb = wp.tile([128, M0 + M1 + M2 + 3], _f32)
        nc.sync.dma_start(cst_sb[:], cst_d[:, :])
        b0_sb = cst_sb[:, 0:M0]
        b1_sb = cst_sb[:, M0:M0 + M1]
        b2_sb = cst_sb[:, M0 + M1:M0 + M1 + M2]
        kv_sb = cst_sb[:, M0 + M1 + M2:M0 + M1 + M2 + 3]
        nc.sync.dma_start(w0_sb[:, :, 512:1024], w0_r[:, :, 512:1024])
        id_sb = wp.tile([16, 16], _f32)
        w1_sb = wp.tile([128, K8, DEEP[1]], _f8)
        w2_sb = wp.tile([128, M1, DEEP[2]], _f8)
        owd_sb = wp.tile([128, M2, 16], _bf)

        def _late_loads():
            nc.sync.dma_start(w1_sb[:], w1_d[:, :].rearrange("p (k m) -> p k m", k=K8))
            nc.sync.dma_start(w2_sb[:], w2_d[:, :].rearrange("p (k m) -> p k m", k=M1))
            nc.sync.dma_start(owd_sb[:], owd_d[:, :].rearrange("p (k c) -> p k c", k=M2))
            nc.sync.dma_start(id_sb[:], id_d[:, :])

        # "Observe" ops: each engine touches its DMA-loaded constants once so
        # steady-state instructions carry at most one semaphore wait.
        obs = wp.tile([128, 8], _f32)
        nc.vector.tensor_copy(obs[:, 0:1], kv_sb[:, 0:1])
        nc.gpsimd.tensor_copy(obs[:, 4:5], kv_sb[:, 0:1])
        nc.scalar.activation(obs[:, 1:2], b0_sb[:, 0:1], AF.Copy)
        nc.scalar.activation(obs[:, 2:3], b1_sb[:, 0:1], AF.Copy)
        nc.scalar.activation(obs[:, 3:4], b2_sb[:, 0:1], AF.Copy)
        # PE warm-up burst: keep the PE busy during the startup DMA window so
        # the clock p-state ramps before the first real matmul group.
        warm = wp.tile([128, 512], _bf)
        nc.gpsimd.memset(warm[:], 0.0)
        warm_ps = dps.tile([128, 512], _f32, tag="dps", name="warm_ps")
        for _ in range(NWARM):
            nc.tensor.matmul(
                warm_ps[:], lhsT=warm[:, 0:128], rhs=warm[:], start=True, stop=True
            )
        for w_ap in (pq8_sb[:, 0, 0:1], w0_sb[:, 0, 0:1]):
            nc.tensor.matmul(
                warm_ps[0:1, 0:1], lhsT=w_ap, rhs=w_ap, start=True, stop=True
            )

        # ---- per-chunk stage emitters ----
        x8ts = {0: x8t0}
        y0ts, y1ts, y2ts, qts, sbqs = {}, {}, {}, {}, {}

        def dma_x(c):
            x8t = x8p.tile([128, KR, CHUNK], _f8, tag="x8", name=f"x8_{c}")
            if SPLIT_DMA:
                nc.sync.dma_start(x8t[:, 0:K8, :], x8r[:, c, 0:K8, :])
                nc.sync.dma_start(x8t[:, K8:KR, :], x8r[:, c, K8:KR, :])
            else:
                nc.sync.dma_start(x8t[:], x8r[:, c, :, :])
            x8ts[c] = x8t

        def pq_open(c):
            # P/Q DoubleRow group over hi tiles (P, Q1, Q2) and lo tiles (Q3)
            qt = qps.tile([16, CHUNK], _f32, tag="pq", name=f"pq_{c}")
            qts[c] = qt
            for kp in range(KR // 2):
                nc.tensor.matmul(
                    qt[:],
                    lhsT=pq8_sb[:, 2 * kp:2 * kp + 2, :],
                    rhs=x8ts[c][:, 2 * kp:2 * kp + 2, :],
                    start=(kp == 0),
                    stop=False,
                    perf_mode=PM.DoubleRow,
                    skip_group_check=True,
                )

        def l0_alloc(c):
            y0ts[c] = y0p.tile([128, K8, CHUNK], _f8, tag="y0", name=f"y0_{c}")

        def l0(c, m):
            ps = dps.tile([128, CHUNK], _f32, tag="dps", name=f"ps0_{c}_{m}")
            for kp in range(K8 // 2):
                nc.tensor.matmul(
                    ps[:],
                    lhsT=w0_sb[:, 2 * kp:2 * kp + 2, m * 128:(m + 1) * 128],
                    rhs=x8ts[c][:, 2 * kp:2 * kp + 2, :],
                    start=(kp == 0),
                    stop=(kp == K8 // 2 - 1),
                    perf_mode=PM.DoubleRow,
                )
            # drain-balance: even m on ACT, odd m on DVE (zero-bias form)
            if zb and m in Y0_DVE:
                nc.vector.tensor_scalar(
                    y0ts[c][:, m, :], ps[:], SCL0, 0.0, op0=OP.mult, op1=OP.max
                )
            else:
                nc.scalar.activation(
                    y0ts[c][:, m, :], ps[:], AF.Relu, bias=b0_sb[:, m:m + 1], scale=SCL0
                )

        def l1(c):
            y1t = y1p.tile([128, M1, CHUNK], _f8, tag="y1", name=f"y1_{c}")
            y1ts[c] = y1t
            for m in range(M1):
                ps = dps.tile([128, CHUNK], _f32, tag="dps", name=f"ps1_{c}_{m}")
                for kp in range(K8 // 2):
                    nc.tensor.matmul(
                        ps[:],
                        lhsT=w1_sb[:, 2 * kp:2 * kp + 2, m * 128:(m + 1) * 128],
                        rhs=y0ts[c][:, 2 * kp:2 * kp + 2, :],
                        start=(kp == 0),
                        stop=(kp == K8 // 2 - 1),
                        perf_mode=PM.DoubleRow,
                    )
                if zb and m in Y1_DVE:
                    nc.vector.tensor_scalar(
                        y1t[:, m, :], ps[:], SCL1, 0.0, op0=OP.mult, op1=OP.max
                    )
                else:
                    nc.scalar.activation(
                        y1t[:, m, :], ps[:], AF.Relu, bias=b1_sb[:, m:m + 1], scale=SCL1
                    )

        def l2(c):
            y2t = y2p.tile([128, M2, CHUNK], _bf, tag="y2", name=f"y2_{c}")
            y2ts[c] = y2t
            for m in range(M2):
                ps = dps.tile([128, CHUNK], _f32, tag="dps", name=f"ps2_{c}_{m}")
                for kp in range(M1 // 2):
                    nc.tensor.matmul(
                        ps[:],
                        lhsT=w2_sb[:, 2 * kp:2 * kp + 2, m * 128:(m + 1) * 128],
                        rhs=y1ts[c][:, 2 * kp:2 * kp + 2, :],
                        start=(kp == 0),
                        stop=(kp == M1 // 2 - 1),
                        perf_mode=PM.DoubleRow,
                    )
                if zb and m in Y2_DVE:
                    nc.vector.tensor_scalar(
                        y2t[:, m, :], ps[:], SCL2, 0.0, op0=OP.mult, op1=OP.max
                    )
                else:
                    nc.scalar.activation(
                        y2t[:, m, :], ps[:], AF.Relu, bias=b2_sb[:, m:m + 1],
                        scale=SCL2,
                    )

        def out_mv(c, j):
            # deep output matvec accumulates into row 6 of the P/Q group
            nc.tensor.matmul(
                qts[c][:],
                lhsT=owd_sb[:, j, :],
                rhs=y2ts[c][:, j, :],
                start=False,
                stop=(j == M2 - 1),
                skip_group_check=True,
            )
            if j == M2 - 1:
                sbq = pqs.tile([16, CHUNK], _f32, tag="sbq", name=f"sbq_{c}")
                if SBQ_DVE:
                    nc.vector.tensor_copy(sbq[:], qts[c][:])
                else:
                    nc.scalar.activation(sbq[:], qts[c][:], AF.Copy)
                sbqs[c] = sbq

        def tail(c):
            sbq = sbqs[c]
            # transpose scratch borrows a [128,512] slot from the dps ring
            pt = dps.tile([128, CHUNK], _f32, tag="dps", name=f"ptr_{c}")
            ptr = pt[:, 0:64].rearrange("p (s i) -> p s i", s=4)
            for s in range(4):
                nc.tensor.transpose(
                    ptr[:, s, :], sbq[:, s * 128:(s + 1) * 128], id_sb[:]
                )
            # one small ACT copy frees the PSUM slot; the scalar combine then
            # runs entirely on the otherwise-idle Pool engine from SBUF.
            # pb rows: 0-2 P_i, 3 Q1 (hi.hi), 4 Q2 (hi.lo), 5 Q3 (lo.hi),
            # 6 Rdeep.  The a-chain carries the C_Q fold: t1 = (1+P0)*C_Q.
            pb = rp.tile([128, 4, 16], _f32, tag="pb", name=f"pb_{c}")
            nc.scalar.activation(pb[:], ptr[:], AF.Copy)
            t1 = rp.tile([128, 4], _f32, tag="t1", name=f"t1_{c}")
            t2 = rp.tile([128, 4], _f32, tag="t2", name=f"t2_{c}")
            t3 = rp.tile([128, 4], _f32, tag="t3", name=f"t3_{c}")
            nc.gpsimd.tensor_scalar(
                t1[:], pb[:, :, 0], S_P * C_Q, C_Q, op0=OP.mult, op1=OP.add
            )
            nc.gpsimd.tensor_scalar(
                t2[:], pb[:, :, 1], S_P, 1.0, op0=OP.mult, op1=OP.add
            )
            nc.gpsimd.tensor_scalar(
                t3[:], pb[:, :, 2], S_P, 1.0, op0=OP.mult, op1=OP.add
            )
            u = rp.tile([128, 4], _f32, tag="u", name=f"u_{c}")
            nc.gpsimd.tensor_tensor(out=u[:], in0=pb[:, :, 4], in1=pb[:, :, 5], op=OP.add)
            qv = rp.tile([128, 4], _f32, tag="qv", name=f"qv_{c}")
            nc.gpsimd.tensor_scalar(qv[:], u[:], 1.0 / 16.0, None, op0=OP.mult)
            nc.gpsimd.tensor_tensor(out=qv[:], in0=qv[:], in1=pb[:, :, 3], op=OP.add)
            acc = rp.tile([128, 4], _f32, tag="acc", name=f"acc_{c}")
            nc.gpsimd.tensor_tensor(out=acc[:], in0=t1[:], in1=t2[:], op=OP.mult)
            if not zb:
                nc.gpsimd.tensor_scalar_add(acc[:], acc[:], kv_sb[:, 0:1])
            nc.gpsimd.tensor_tensor(out=acc[:], in0=acc[:], in1=t3[:], op=OP.mult)
            if not zb:
                nc.gpsimd.tensor_scalar_add(acc[:], acc[:], kv_sb[:, 1:2])
            nc.gpsimd.tensor_tensor(out=acc[:], in0=acc[:], in1=qv[:], op=OP.mult)
            res = rp.tile([128, 4], _f32, tag="res", name=f"res_{c}")
            if zk:
                nc.gpsimd.tensor_tensor(out=res[:], in0=acc[:], in1=pb[:, :, 6], op=OP.add)
            else:
                nc.gpsimd.tensor_tensor(out=acc[:], in0=acc[:], in1=pb[:, :, 6], op=OP.add)
                nc.gpsimd.tensor_scalar_add(res[:], acc[:], kv_sb[:, 2:3])
            nc.sync.dma_start(
                out=out_d[c * CHUNK:(c + 1) * CHUNK, :].rearrange(
                    "(s p) o -> p (s o)", p=128
                ),
                in_=res[:],
            )

        # ---- software-pipelined main loop ----
        for it in range(NCHUNK):
            A, Bc, Cc, Tc = it, it - 1, it - 2, it - 3
            if A == 0:
                l0_alloc(0)
                for m in range(4):
                    l0(0, m)
                pq_open(0)
                for m in range(4, M0):
                    l0(0, m)
                dma_x(1)
                _late_loads()
                continue
            if 0 <= Cc:
                l2(Cc)
            if A + 1 < NCHUNK:
                dma_x(A + 1)
            l0_alloc(A)
            l0(A, 0)
            if 0 <= Tc:
                tail(Tc)
            l0(A, 1)
            if EARLY_OUT and 0 <= Cc:
                out_mv(Cc, 0)
                out_mv(Cc, 1)
            pq_open(A)
            l0(A, 2)
            l0(A, 3)
            if 0 <= Bc:
                l1(Bc)
            for m in range(4, M0):
                l0(A, m)
            # last chunk: L1 inline (PE is idle in the drain anyway)
            if INLINE_L1 and A == NCHUNK - 1:
                l1(A)
            if not EARLY_OUT and 0 <= Cc:
                out_mv(Cc, 0)
                out_mv(Cc, 1)
        # ---- compressed drain: PE is idle, so collapse the stage skew ----
        for step in DRAIN_SEQ:
            kind, c = step
            if kind == "l2":
                l2(c)
            elif kind == "out":
                out_mv(c, 0)
                out_mv(c, 1)
            elif kind == "l1":
                l1(c)
            elif kind == "tail":
                tail(c)

    nc.compile()
    return nc


def _get_nc(zb=True, zk=True):
    key = f"nc_zb{int(zb)}_zk{int(zk)}"
    if key not in _CACHE:
        _CACHE[key] = _build_nc(zb=zb, zk=zk)
    return _CACHE[key]


def _prep_in_maps(inputs, zb):
    fi = np.asarray(inputs["feature_index"]).astype(np.int64)
    fvv = np.asarray(inputs["feature_value"], dtype=np.float32)
    emb = np.asarray(inputs["emb_table"], dtype=np.float32)
    cw = np.asarray(inputs["cross_w"], dtype=np.float32)
    cb = np.asarray(inputs["cross_b"], dtype=np.float32)
    w0 = np.asarray(inputs["w0"], dtype=np.float32)
    b0 = np.asarray(inputs["b0"], dtype=np.float32)
    w1 = np.asarray(inputs["w1"], dtype=np.float32)
    b1 = np.asarray(inputs["b1"], dtype=np.float32)
    w2 = np.asarray(inputs["w2"], dtype=np.float32)
    b2 = np.asarray(inputs["b2"], dtype=np.float32)
    ow = np.asarray(inputs["out_w"], dtype=np.float32).reshape(-1)
    ob = np.asarray(inputs["out_b"], dtype=np.float32).reshape(-1)

    # host gather with feature_value premultiplied (f32, before any cast)
    x = emb[fi] * fvv[:, :, None]               # [B, F, E] f32
    x = x.reshape(B, D)

    # hi/lo fp8 split of x (lo = 16x the hi-quantization residual)
    xs = np.zeros((B, D8), dtype=np.float32)
    xs[:, :D] = x * SX
    xhi = xs.astype(_np_f8)
    xlo = ((xs - xhi.astype(np.float32)) * 16.0).astype(_np_f8)

    # shared (replicated) weights
    w0p = np.zeros((D8, DEEP[0]), dtype=np.float32)
    w0p[:D] = w0 * SW0
    w0q = np.ascontiguousarray(
        w0p.reshape(K8, 128, DEEP[0]).transpose(1, 0, 2).reshape(128, -1)
    ).astype(_np_f8)
    w1q = np.ascontiguousarray(
        (w1 * SW1).reshape(K8, 128, DEEP[1]).transpose(1, 0, 2).reshape(128, -1)
    ).astype(_np_f8)
    w2q = np.ascontiguousarray(
        (w2 * SW2).reshape(M1, 128, DEEP[2]).transpose(1, 0, 2).reshape(128, -1)
    ).astype(_np_f8)

    # P/Q lhsT: hi k-tiles get [cw0 cw1 cw2 owch owcl 0 0 0]; lo k-tiles get
    # owch in column 5 (-> Q3).  All quantized fp8.
    owc = np.zeros((D8,), dtype=np.float32)
    owc[:D] = ow[:D]
    owch = (owc * SQC).astype(_np_f8)
    owcl = ((owc * SQC - owch.astype(np.float32)) * 16.0).astype(_np_f8)
    cwp = np.zeros((N_CROSS, D8), dtype=np.float32)
    cwp[:, :D] = cw * SCW
    pq8 = np.zeros((128, KR, 16), dtype=_np_f8)
    for k in range(K8):
        seg = slice(k * 128, (k + 1) * 128)
        for i in range(N_CROSS):
            pq8[:, k, i] = cwp[i, seg].astype(_np_f8)
        pq8[:, k, 3] = owch[seg]
        pq8[:, k, 4] = owcl[seg]
        pq8[:, K8 + k, 5] = owch[seg]
    pq8 = np.ascontiguousarray(pq8.reshape(128, -1))
    owd = np.zeros((128, M2, 16), dtype=np.float32)
    for j in range(M2):
        owd[:, j, 6] = ow[D + j * 128:D + (j + 1) * 128]
    owd = np.ascontiguousarray(owd.reshape(128, -1)).astype(_np_bf)

    C = np.cumsum(cb)                           # C[i] = cb_0 + ... + cb_i
    W = cw.sum(axis=1)
    k1 = C[0] * W[1] * C_Q
    k2 = C[1] * W[2] * C_Q
    kf = ob[0] + C[2] * ow[:D].sum()
    b0s = (b0 * SY0).reshape(M0, 128).T
    b1s = (b1 * SY1).reshape(M1, 128).T
    b2r = b2.reshape(M2, 128).T
    kv = np.tile(np.array([[k1, k2, kf]], dtype=np.float32), (128, 1))
    cst = np.ascontiguousarray(
        np.concatenate([b0s, b1s, b2r, kv], axis=1).astype(np.float32)
    )
    ident = np.eye(16, dtype=np.float32)

    shared = dict(w0=w0q, w1=w1q, w2=w2q, pq8=pq8, owd=owd, cst=cst, ident=ident)

    in_maps = []
    for core in range(N_CORES):
        rows = slice(core * S, (core + 1) * S)
        # per-chunk layout [128, c, k(hi 0-7, lo 8-15), j]
        xh = xhi[rows].reshape(NCHUNK, CHUNK, K8, 128).transpose(3, 0, 2, 1)
        xl = xlo[rows].reshape(NCHUNK, CHUNK, K8, 128).transpose(3, 0, 2, 1)
        x8 = np.concatenate([xh, xl], axis=2).reshape(128, -1)
        in_maps.append(dict(x8=np.ascontiguousarray(x8), **shared))
    return in_maps


def _zb(inputs):
    return not (
        np.any(np.asarray(inputs["b0"])) or np.any(np.asarray(inputs["b1"]))
        or np.any(np.asarray(inputs["b2"]))
    )


def _kf(inputs):
    cb = np.asarray(inputs["cross_b"], dtype=np.float32)
    ow = np.asarray(inputs["out_w"], dtype=np.float32).reshape(-1)
    ob = np.asarray(inputs["out_b"], dtype=np.float32).reshape(-1)
    return float(ob[0] + np.cumsum(cb)[2] * ow[:D].sum())


def _run(inputs, trace=False, **kw):
    zb = _zb(inputs)
    nc = _get_nc(zb=zb, zk=(_kf(inputs) == 0.0))
    in_maps = _prep_in_maps(inputs, zb)
    res = run_bass_kernel_spmd(
        nc, in_maps, core_ids=list(range(N_CORES)), trace=trace, **kw
    )
    out = np.concatenate([r["out"] for r in res.results], axis=0)
    return out.astype(np.float32), res


def kernel(**inputs) -> np.ndarray:
    out, _ = _run(inputs, trace=False)
    return out


# revision 35
# speedup vs baseline: 1.0168x; 1.0111x over previous
"""DCN (cross+deep) Trainium2 Bass kernel, 8 NeuronCores.

Sharding: data-parallel over batch (2048 rows/core); embedding rows gathered
host-side (feature_value premultiplied in f32), cross/deep weights replicated.

Math restructure (exact): the cross tower never needs materializing. Since
  y_{i+1} = x0 * (y_i . w_i) + cb_i + y_i
preserves the form y_i = x0 * a_i + C_i (a_i per-row scalar, C_i = cumsum cb),
the whole cross branch + its slice of the output dot reduces to per-row
scalars P_i = x0 . w_i and Q = x0 . ow_cross:
  a_1 = 1 + P_0;  a_{i+1} = a_i (1 + P_i) + C_i W_i   (W_i = sum w_i)
  r_cross = a_3 Q + C_3 sum(ow_cross)
The P/Q pass, deep output matvec (row 6) and everything else accumulate in
one narrow PSUM group; a tiny PE transpose turns [8, 512] into per-row
scalars and the final combine runs on the otherwise-idle Pool engine.

Everything heavy runs in fp8e4m3 with DoubleRow perf mode (0.5 PE
cycles/row, two k-tiles per call): L0 (1024-padded x 1024), L1 (1024x512),
L2 (512x256) and the P/Q pass. Accuracy (gate 2e-2, measured 1.4e-2) is
held by residual compensation where it matters: Q = x . ow_cross feeds the
output directly, so it is computed as xhi.owh + (xhi.owl + xlo.owh)/16
(7-bit effective mantissas); P_i only enter via (1+P_i) ~ 1, so plain fp8
is fine. Host pre-quantizes x (*64, + *16 residual) and weights (*16); ACT
and DVE tensor_scalar ops fuse dequant+relu+requant via scale folding.

Schedule: software-pipelined across chunks with stage skew so the PE never
waits on ACT/DVE drain latency: iteration `it` runs L2+out for chunk it-2,
P/Q+L0 for chunk it, L1 for chunk it-1, the chunk it-3 tail (transpose +
Pool combine + store) interleaved, and chunk it+1's x DMA prefetched.
PSUM drains alternate ACT/DVE per tile to balance the two engines.
"""

import numpy as np
import ml_dtypes
from contextlib import ExitStack

import concourse.tile as tile
import concourse.mybir as mybir
from concourse import bacc
from concourse.bass_utils import run_bass_kernel_spmd

# ---- problem constants (hardcoded; kernel.py must be self-contained) ----
B, F, E = 16384, 26, 32
NF = 1_000_000
D = F * E                    # 832
DEEP = (1024, 512, 256)
N_CROSS = 3
N_CORES = 8
S = B // N_CORES             # 2048 batch rows per core
CHUNK = 512
NCHUNK = S // CHUNK          # 4
K8 = 8                       # fp8 k-tiles (1024 pad of 832)
KR = 2 * K8                  # hi + lo x tiles per chunk
D8 = K8 * 128                # 1024
M0, M1, M2 = DEEP[0] // 128, DEEP[1] // 128, DEEP[2] // 128  # 8, 4, 2

# fp8 scaling (powers of two; folded into ACT/DVE scales and Pool combine)
SX, SW0, SY0, SW1 = 64.0, 16.0, 64.0, 16.0
SY1, SW2 = 64.0, 16.0
SCW, SQC = 16.0, 16.0
SCL0 = SY0 / (SX * SW0)      # PSUM0 -> sy0*y0
SCL1 = SY1 / (SY0 * SW1)     # PSUM1 -> sy1*y1
SCL2 = 1.0 / (SY1 * SW2)     # PSUM2 -> y2 (natural)
C_Q = 1.0 / (SX * SQC)       # Q dequant (folded into the a-chain)
S_P = 1.0 / (SX * SCW)       # P dequant

NWARM = 8                    # PE p-state warm-up matmuls (tuned by sweep)
Y0_DVE = (1, 3, 5, 7)        # which y0 m-tiles drain on DVE (zb path)
Y1_DVE = (1, 3)
Y2_DVE = (0,)
SBQ_DVE = False
SPLIT_DMA = True
INLINE_L1 = False
EARLY_OUT = False
DPS_BUFS = 5
QPS_BUFS = 3
DRAIN_SEQ = (                # drain ordering (tuned by sweep)
    ("l2", 2), ("l1", 3), ("out", 2), ("tail", 1),
    ("l2", 3), ("out", 3), ("tail", 2), ("tail", 3),
)

_bf = mybir.dt.bfloat16
_f32 = mybir.dt.float32
_f8 = mybir.dt.float8e4
_np_bf = ml_dtypes.bfloat16
_np_f8 = ml_dtypes.float8_e4m3

_CACHE = {}


def _build_nc(zb=True, zk=True):
    """zb: all of b0/b1/b2 are zero -> half the relu drains use DVE
    2-op tensor_scalar forms; otherwise every drain runs on ACT with bias."""
    AF = mybir.ActivationFunctionType
    OP = mybir.AluOpType
    PM = mybir.MatmulPerfMode
    nc = bacc.Bacc(
        "TRN2", target_bir_lowering=False, debug=False, num_devices=N_CORES
    )

    x8_d = nc.dram_tensor("x8", [128, NCHUNK * KR * CHUNK], _f8, kind="ExternalInput")
    w0_d = nc.dram_tensor("w0", [128, K8 * DEEP[0]], _f8, kind="ExternalInput")
    w1_d = nc.dram_tensor("w1", [128, K8 * DEEP[1]], _f8, kind="ExternalInput")
    w2_d = nc.dram_tensor("w2", [128, M1 * DEEP[2]], _f8, kind="ExternalInput")
    pq8_d = nc.dram_tensor("pq8", [128, KR * 16], _f8, kind="ExternalInput")
    owd_d = nc.dram_tensor("owd", [128, M2 * 16], _bf, kind="ExternalInput")
    id_d = nc.dram_tensor("ident", [16, 16], _f32, kind="ExternalInput")
    # f32 consts: [b0*sy0 (8) | b1*sy1 (4) | b2 (2) | k1*cq k2*cq kf (3)]
    cst_d = nc.dram_tensor("cst", [128, M0 + M1 + M2 + 3], _f32, kind="ExternalInput")
    out_d = nc.dram_tensor("out", [S, 1], _f32, kind="ExternalOutput")

    with ExitStack() as ctx:
        tc = ctx.enter_context(tile.TileContext(nc))
        wp = ctx.enter_context(tc.tile_pool(name="wp", bufs=1))
        x8p = ctx.enter_context(tc.tile_pool(name="x8p", bufs=2))
        y0p = ctx.enter_context(tc.tile_pool(name="y0p", bufs=2))
        y1p = ctx.enter_context(tc.tile_pool(name="y1p", bufs=2))
        y2p = ctx.enter_context(tc.tile_pool(name="y2p", bufs=2))
        pqs = ctx.enter_context(tc.tile_pool(name="pqs", bufs=2))
        rp = ctx.enter_context(tc.tile_pool(name="rp", bufs=2))
        dps = ctx.enter_context(tc.tile_pool(name="dps", bufs=DPS_BUFS, space="PSUM"))
        qps = ctx.enter_context(tc.tile_pool(name="qps", bufs=QPS_BUFS, space="PSUM"))

        # ---- weights / constants to SBUF ----
        # DMA emission order == need order: x8_0 + w0 feed the first L0
        # groups (behind the warm-up burst); w1/w2/owd/ident load late.
        w0_sb = wp.tile([128, K8, DEEP[0]], _f8)
        w0_r = w0_d[:, :].rearrange("p (k m) -> p k m", k=K8)
        x8r = x8_d[:, :].rearrange("p (c k j) -> p c k j", c=NCHUNK, k=KR)
        x8t0 = x8p.tile([128, KR, CHUNK], _f8, tag="x8", name="x8_0")
        # k-tiles 7 (hi pad) and 15 (lo pad) are all-zero: memset on the idle
        # Pool engine instead of streaming zeros over DMA
        nc.gpsimd.memset(x8t0[:, K8 - 1, :], 0.0)
        nc.gpsimd.memset(x8t0[:, KR - 1, :], 0.0)
        nc.sync.dma_start(x8t0[:, 0:K8 - 1, :], x8r[:, 0, 0:K8 - 1, :])
        nc.sync.dma_start(w0_sb[:, :, 0:512], w0_r[:, :, 0:512])
        nc.sync.dma_start(x8t0[:, K8:KR - 1, :], x8r[:, 0, K8:KR - 1, :])
        pq8_sb = wp.tile([128, KR, 16], _f8)
        nc.sync.dma_start(pq8_sb[:], pq8_d[:, :].rearrange("p (k c) -> p k c", k=KR))
        cst_sb = wp.tile([128, M0 + M1 + M2 + 3], _f32)
        nc.sync.dma_start(cst_sb[:], cst_d[:, :])
        b0_sb = cst_sb[:, 0:M0]
        b1_sb = cst_sb[:, M0:M0 + M1]
        b2_sb = cst_sb[:, M0 + M1:M0 + M1 + M2]
        kv_sb = cst_sb[:, M0 + M1 + M2:M0 + M1 + M2 + 3]
        nc.sync.dma_start(w0_sb[:, :, 512:1024], w0_r[:, :, 512:1024])
        id_sb = wp.tile([16, 16], _f32)
        w1_sb = wp.tile([128, K8, DEEP[1]], _f8)
        w2_sb = wp.tile([128, M1, DEEP[2]], _f8)
        owd_sb = wp.tile([128, M2, 16], _bf)

        def _late_loads():
            nc.sync.dma_start(w1_sb[:], w1_d[:, :].rearrange("p (k m) -> p k m", k=K8))
            nc.sync.dma_start(w2_sb[:], w2_d[:, :].rearrange("p (k m) -> p k m", k=M1))
            nc.sync.dma_start(owd_sb[:], owd_d[:, :].rearrange("p (k c) -> p k c", k=M2))
            nc.sync.dma_start(id_sb[:], id_d[:, :])

        # "Observe" ops: each engine touches its DMA-loaded constants once so
        # steady-state instructions carry at most one semaphore wait.
        obs = wp.tile([128, 8], _f32)
        nc.vector.tensor_copy(obs[:, 0:1], kv_sb[:, 0:1])
        nc.gpsimd.tensor_copy(obs[:, 4:5], kv_sb[:, 0:1])
        nc.scalar.activation(obs[:, 1:2], b0_sb[:, 0:1], AF.Copy)
        nc.scalar.activation(obs[:, 2:3], b1_sb[:, 0:1], AF.Copy)
        nc.scalar.activation(obs[:, 3:4], b2_sb[:, 0:1], AF.Copy)
        # PE warm-up burst: keep the PE busy during the startup DMA window so
        # the clock p-state ramps before the first real matmul group.
        warm = wp.tile([128, 512], _bf)
        nc.gpsimd.memset(warm[:], 0.0)
        warm_ps = dps.tile([128, 512], _f32, tag="dps", name="warm_ps")
        for _ in range(NWARM):
            nc.tensor.matmul(
                warm_ps[:], lhsT=warm[:, 0:128], rhs=warm[:], start=True, stop=True
            )
        for w_ap in (pq8_sb[:, 0, 0:1], w0_sb[:, 0, 0:1]):
            nc.tensor.matmul(
                warm_ps[0:1, 0:1], lhsT=w_ap, rhs=w_ap, start=True, stop=True
            )

        # ---- per-chunk stage emitters ----
        x8ts = {0: x8t0}
        y0ts, y1ts, y2ts, qts, sbqs = {}, {}, {}, {}, {}

        def dma_x(c):
            x8t = x8p.tile([128, KR, CHUNK], _f8, tag="x8", name=f"x8_{c}")
            nc.gpsimd.memset(x8t[:, K8 - 1, :], 0.0)
            nc.gpsimd.memset(x8t[:, KR - 1, :], 0.0)
            nc.sync.dma_start(x8t[:, 0:K8 - 1, :], x8r[:, c, 0:K8 - 1, :])
            nc.sync.dma_start(x8t[:, K8:KR - 1, :], x8r[:, c, K8:KR - 1, :])
            x8ts[c] = x8t

        def pq_open(c):
            # P/Q DoubleRow group over hi tiles (P, Q1, Q2) and lo tiles (Q3)
            qt = qps.tile([16, CHUNK], _f32, tag="pq", name=f"pq_{c}")
            qts[c] = qt
            for kp in range(KR // 2):
                nc.tensor.matmul(
                    qt[:],
                    lhsT=pq8_sb[:, 2 * kp:2 * kp + 2, :],
                    rhs=x8ts[c][:, 2 * kp:2 * kp + 2, :],
                    start=(kp == 0),
                    stop=False,
                    perf_mode=PM.DoubleRow,
                    skip_group_check=True,
                )

        def l0_alloc(c):
            y0ts[c] = y0p.tile([128, K8, CHUNK], _f8, tag="y0", name=f"y0_{c}")

        def l0(c, m):
            ps = dps.tile([128, CHUNK], _f32, tag="dps", name=f"ps0_{c}_{m}")
            for kp in range(K8 // 2):
                nc.tensor.matmul(
                    ps[:],
                    lhsT=w0_sb[:, 2 * kp:2 * kp + 2, m * 128:(m + 1) * 128],
                    rhs=x8ts[c][:, 2 * kp:2 * kp + 2, :],
                    start=(kp == 0),
                    stop=(kp == K8 // 2 - 1),
                    perf_mode=PM.DoubleRow,
                )
            # drain-balance: even m on ACT, odd m on DVE (zero-bias form)
            if zb and m in Y0_DVE:
                nc.vector.tensor_scalar(
                    y0ts[c][:, m, :], ps[:], SCL0, 0.0, op0=OP.mult, op1=OP.max
                )
            else:
                nc.scalar.activation(
                    y0ts[c][:, m, :], ps[:], AF.Relu, bias=b0_sb[:, m:m + 1], scale=SCL0
                )

        def l1(c):
            y1t = y1p.tile([128, M1, CHUNK], _f8, tag="y1", name=f"y1_{c}")
            y1ts[c] = y1t
            for m in range(M1):
                ps = dps.tile([128, CHUNK], _f32, tag="dps", name=f"ps1_{c}_{m}")
                for kp in range(K8 // 2):
                    nc.tensor.matmul(
                        ps[:],
                        lhsT=w1_sb[:, 2 * kp:2 * kp + 2, m * 128:(m + 1) * 128],
                        rhs=y0ts[c][:, 2 * kp:2 * kp + 2, :],
                        start=(kp == 0),
                        stop=(kp == K8 // 2 - 1),
                        perf_mode=PM.DoubleRow,
                    )
                if zb and m in Y1_DVE:
                    nc.vector.tensor_scalar(
                        y1t[:, m, :], ps[:], SCL1, 0.0, op0=OP.mult, op1=OP.max
                    )
                else:
                    nc.scalar.activation(
                        y1t[:, m, :], ps[:], AF.Relu, bias=b1_sb[:, m:m + 1], scale=SCL1
                    )

        def l2(c):
            y2t = y2p.tile([128, M2, CHUNK], _bf, tag="y2", name=f"y2_{c}")
            y2ts[c] = y2t
            for m in range(M2):
                ps = dps.tile([128, CHUNK], _f32, tag="dps", name=f"ps2_{c}_{m}")
                for kp in range(M1 // 2):
                    nc.tensor.matmul(
                        ps[:],
                        lhsT=w2_sb[:, 2 * kp:2 * kp + 2, m * 128:(m + 1) * 128],
                        rhs=y1ts[c][:, 2 * kp:2 * kp + 2, :],
                        start=(kp == 0),
                        stop=(kp == M1 // 2 - 1),
                        perf_mode=PM.DoubleRow,
                    )
                if zb and m in Y2_DVE:
                    nc.vector.tensor_scalar(
                        y2t[:, m, :], ps[:], SCL2, 0.0, op0=OP.mult, op1=OP.max
                    )
                else:
                    nc.scalar.activation(
                        y2t[:, m, :], ps[:], AF.Relu, bias=b2_sb[:, m:m + 1],
                        scale=SCL2,
                    )

        def out_mv(c, j):
            # deep output matvec accumulates into row 6 of the P/Q group
            nc.tensor.matmul(
                qts[c][:],
                lhsT=owd_sb[:, j, :],
                rhs=y2ts[c][:, j, :],
                start=False,
                stop=(j == M2 - 1),
                skip_group_check=True,
            )
            if j == M2 - 1:
                sbq = pqs.tile([16, CHUNK], _f32, tag="sbq", name=f"sbq_{c}")
                if SBQ_DVE:
                    nc.vector.tensor_copy(sbq[:], qts[c][:])
                else:
                    nc.scalar.activation(sbq[:], qts[c][:], AF.Copy)
                sbqs[c] = sbq

        def tail(c):
            sbq = sbqs[c]
            # transpose scratch borrows a [128,512] slot from the dps ring
            pt = dps.tile([128, CHUNK], _f32, tag="dps", name=f"ptr_{c}")
            ptr = pt[:, 0:64].rearrange("p (s i) -> p s i", s=4)
            for s in range(4):
                nc.tensor.transpose(
                    ptr[:, s, :], sbq[:, s * 128:(s + 1) * 128], id_sb[:]
                )
            # one small ACT copy frees the PSUM slot; the scalar combine then
            # runs entirely on the otherwise-idle Pool engine from SBUF.
            # pb rows: 0-2 P_i, 3 Q1 (hi.hi), 4 Q2 (hi.lo), 5 Q3 (lo.hi),
            # 6 Rdeep.  The a-chain carries the C_Q fold: t1 = (1+P0)*C_Q.
            pb = rp.tile([128, 4, 16], _f32, tag="pb", name=f"pb_{c}")
            nc.scalar.activation(pb[:], ptr[:], AF.Copy)
            t1 = rp.tile([128, 4], _f32, tag="t1", name=f"t1_{c}")
            t2 = rp.tile([128, 4], _f32, tag="t2", name=f"t2_{c}")
            t3 = rp.tile([128, 4], _f32, tag="t3", name=f"t3_{c}")
            nc.gpsimd.tensor_scalar(
                t1[:], pb[:, :, 0], S_P * C_Q, C_Q, op0=OP.mult, op1=OP.add
            )
            nc.gpsimd.tensor_scalar(
                t2[:], pb[:, :, 1], S_P, 1.0, op0=OP.mult, op1=OP.add
            )
            nc.gpsimd.tensor_scalar(
                t3[:], pb[:, :, 2], S_P, 1.0, op0=OP.mult, op1=OP.add
            )
            u = rp.tile([128, 4], _f32, tag="u", name=f"u_{c}")
            nc.gpsimd.tensor_tensor(out=u[:], in0=pb[:, :, 4], in1=pb[:, :, 5], op=OP.add)
            qv = rp.tile([128, 4], _f32, tag="qv", name=f"qv_{c}")
            nc.gpsimd.tensor_scalar(qv[:], u[:], 1.0 / 16.0, None, op0=OP.mult)
            nc.gpsimd.tensor_tensor(out=qv[:], in0=qv[:], in1=pb[:, :, 3], op=OP.add)
            acc = rp.tile([128, 4], _f32, tag="acc", name=f"acc_{c}")
            nc.gpsimd.tensor_tensor(out=acc[:], in0=t1[:], in1=t2[:], op=OP.mult)
            if not zb:
                nc.gpsimd.tensor_scalar_add(acc[:], acc[:], kv_sb[:, 0:1])
            nc.gpsimd.tensor_tensor(out=acc[:], in0=acc[:], in1=t3[:], op=OP.mult)
            if not zb:
                nc.gpsimd.tensor_scalar_add(acc[:], acc[:], kv_sb[:, 1:2])
            nc.gpsimd.tensor_tensor(out=acc[:], in0=acc[:], in1=qv[:], op=OP.mult)
            res = rp.tile([128, 4], _f32, tag="res", name=f"res_{c}")
            if zk:
                nc.gpsimd.tensor_tensor(out=res[:], in0=acc[:], in1=pb[:, :, 6], op=OP.add)
            else:
                nc.gpsimd.tensor_tensor(out=acc[:], in0=acc[:], in1=pb[:, :, 6], op=OP.add)
                nc.gpsimd.tensor_scalar_add(res[:], acc[:], kv_sb[:, 2:3])
            nc.sync.dma_start(
                out=out_d[c * CHUNK:(c + 1) * CHUNK, :].rearrange(
                    "(s p) o -> p (s o)", p=128
                ),
                in_=res[:],
            )

        # ---- software-pipelined main loop ----
        for it in range(NCHUNK):
            A, Bc, Cc, Tc = it, it - 1, it - 2, it - 3
            if A == 0:
                l0_alloc(0)
                for m in range(4):
                    l0(0, m)
                pq_open(0)
                for m in range(4, M0):
                    l0(0, m)
                dma_x(1)
                _late_loads()
                continue
            if 0 <= Cc:
                l2(Cc)
            if A + 1 < NCHUNK:
                dma_x(A + 1)
            l0_alloc(A)
            l0(A, 0)
            if 0 <= Tc:
                tail(Tc)
            l0(A, 1)
            if EARLY_OUT and 0 <= Cc:
                out_mv(Cc, 0)
                out_mv(Cc, 1)
            pq_open(A)
            l0(A, 2)
            l0(A, 3)
            if 0 <= Bc:
                l1(Bc)
            for m in range(4, M0):
                l0(A, m)
            # last chunk: L1 inline (PE is idle in the drain anyway)
            if INLINE_L1 and A == NCHUNK - 1:
                l1(A)
            if not EARLY_OUT and 0 <= Cc:
                out_mv(Cc, 0)
                out_mv(Cc, 1)
        # ---- compressed drain: PE is idle, so collapse the stage skew ----
        for step in DRAIN_SEQ:
            kind, c = step
            if kind == "l2":
                l2(c)
            elif kind == "out":
                out_mv(c, 0)
                out_mv(c, 1)
            elif kind == "l1":
                l1(c)
            elif kind == "tail":
                tail(c)

    nc.compile()
    return nc


def _get_nc(zb=True, zk=True):
    key = f"nc_zb{int(zb)}_zk{int(zk)}"
    if key not in _CACHE:
        _CACHE[key] = _build_nc(zb=zb, zk=zk)
    return _CACHE[key]


def _prep_in_maps(inputs, zb):
    fi = np.asarray(inputs["feature_index"]).astype(np.int64)
    fvv = np.asarray(inputs["feature_value"], dtype=np.float32)
    emb = np.asarray(inputs["emb_table"], dtype=np.float32)
    cw = np.asarray(inputs["cross_w"], dtype=np.float32)
    cb = np.asarray(inputs["cross_b"], dtype=np.float32)
    w0 = np.asarray(inputs["w0"], dtype=np.float32)
    b0 = np.asarray(inputs["b0"], dtype=np.float32)
    w1 = np.asarray(inputs["w1"], dtype=np.float32)
    b1 = np.asarray(inputs["b1"], dtype=np.float32)
    w2 = np.asarray(inputs["w2"], dtype=np.float32)
    b2 = np.asarray(inputs["b2"], dtype=np.float32)
    ow = np.asarray(inputs["out_w"], dtype=np.float32).reshape(-1)
    ob = np.asarray(inputs["out_b"], dtype=np.float32).reshape(-1)

    # host gather with feature_value premultiplied (f32, before any cast)
    x = emb[fi] * fvv[:, :, None]               # [B, F, E] f32
    x = x.reshape(B, D)

    # hi/lo fp8 split of x (lo = 16x the hi-quantization residual)
    xs = np.zeros((B, D8), dtype=np.float32)
    xs[:, :D] = x * SX
    xhi = xs.astype(_np_f8)
    xlo = ((xs - xhi.astype(np.float32)) * 16.0).astype(_np_f8)

    # shared (replicated) weights
    w0p = np.zeros((D8, DEEP[0]), dtype=np.float32)
    w0p[:D] = w0 * SW0
    w0q = np.ascontiguousarray(
        w0p.reshape(K8, 128, DEEP[0]).transpose(1, 0, 2).reshape(128, -1)
    ).astype(_np_f8)
    w1q = np.ascontiguousarray(
        (w1 * SW1).reshape(K8, 128, DEEP[1]).transpose(1, 0, 2).reshape(128, -1)
    ).astype(_np_f8)
    w2q = np.ascontiguousarray(
        (w2 * SW2).reshape(M1, 128, DEEP[2]).transpose(1, 0, 2).reshape(128, -1)
    ).astype(_np_f8)

    # P/Q lhsT: hi k-tiles get [cw0 cw1 cw2 owch owcl 0 0 0]; lo k-tiles get
    # owch in column 5 (-> Q3).  All quantized fp8.
    owc = np.zeros((D8,), dtype=np.float32)
    owc[:D] = ow[:D]
    owch = (owc * SQC).astype(_np_f8)
    owcl = ((owc * SQC - owch.astype(np.float32)) * 16.0).astype(_np_f8)
    cwp = np.zeros((N_CROSS, D8), dtype=np.float32)
    cwp[:, :D] = cw * SCW
    pq8 = np.zeros((128, KR, 16), dtype=_np_f8)
    for k in range(K8):
        seg = slice(k * 128, (k + 1) * 128)
        for i in range(N_CROSS):
            pq8[:, k, i] = cwp[i, seg].astype(_np_f8)
        pq8[:, k, 3] = owch[seg]
        pq8[:, k, 4] = owcl[seg]
        pq8[:, K8 + k, 5] = owch[seg]
    pq8 = np.ascontiguousarray(pq8.reshape(128, -1))
    owd = np.zeros((128, M2, 16), dtype=np.float32)
    for j in range(M2):
        owd[:, j, 6] = ow[D + j * 128:D + (j + 1) * 128]
    owd = np.ascontiguousarray(owd.reshape(128, -1)).astype(_np_bf)

    C = np.cumsum(cb)                           # C[i] = cb_0 + ... + cb_i
    W = cw.sum(axis=1)
    k1 = C[0] * W[1] * C_Q
    k2 = C[1] * W[2] * C_Q
    kf = ob[0] + C[2] * ow[:D].sum()
    b0s = (b0 * SY0).reshape(M0, 128).T
    b1s = (b1 * SY1).reshape(M1, 128).T
    b2r = b2.reshape(M2, 128).T
    kv = np.tile(np.array([[k1, k2, kf]], dtype=np.float32), (128, 1))
    cst = np.ascontiguousarray(
        np.concatenate([b0s, b1s, b2r, kv], axis=1).astype(np.float32)
    )
    ident = np.eye(16, dtype=np.float32)

    shared = dict(w0=w0q, w1=w1q, w2=w2q, pq8=pq8, owd=owd, cst=cst, ident=ident)

    in_maps = []
    for core in range(N_CORES):
        rows = slice(core * S, (core + 1) * S)
        # per-chunk layout [128, c, k(hi 0-7, lo 8-15), j]
        xh = xhi[rows].reshape(NCHUNK, CHUNK, K8, 128).transpose(3, 0, 2, 1)
        xl = xlo[rows].reshape(NCHUNK, CHUNK, K8, 128).transpose(3, 0, 2, 1)
        x8 = np.concatenate([xh, xl], axis=2).reshape(128, -1)
        in_maps.append(dict(x8=np.ascontiguousarray(x8), **shared))
    return in_maps


def _zb(inputs):
    return not (
        np.any(np.asarray(inputs["b0"])) or np.any(np.asarray(inputs["b1"]))
        or np.any(np.asarray(inputs["b2"]))
    )


def _kf(inputs):
    cb = np.asarray(inputs["cross_b"], dtype=np.float32)
    ow = np.asarray(inputs["out_w"], dtype=np.float32).reshape(-1)
    ob = np.asarray(inputs["out_b"], dtype=np.float32).reshape(-1)
    return float(ob[0] + np.cumsum(cb)[2] * ow[:D].sum())


def _run(inputs, trace=False, **kw):
    zb = _zb(inputs)
    nc = _get_nc(zb=zb, zk=(_kf(inputs) == 0.0))
    in_maps = _prep_in_maps(inputs, zb)
    res = run_bass_kernel_spmd(
        nc, in_maps, core_ids=list(range(N_CORES)), trace=trace, **kw
    )
    out = np.concatenate([r["out"] for r in res.results], axis=0)
    return out.astype(np.float32), res


def kernel(**inputs) -> np.ndarray:
    out, _ = _run(inputs, trace=False)
    return out


# revision 36
# speedup vs baseline: 1.0260x; 1.0090x over previous
"""DCN (cross+deep) Trainium2 Bass kernel, 8 NeuronCores.

Sharding: data-parallel over batch (2048 rows/core); embedding rows gathered
host-side (feature_value premultiplied in f32), cross/deep weights replicated.

Math restructure (exact): the cross tower never needs materializing. Since
  y_{i+1} = x0 * (y_i . w_i) + cb_i + y_i
preserves the form y_i = x0 * a_i + C_i (a_i per-row scalar, C_i = cumsum cb),
the whole cross branch + its slice of the output dot reduces to per-row
scalars P_i = x0 . w_i and Q = x0 . ow_cross:
  a_1 = 1 + P_0;  a_{i+1} = a_i (1 + P_i) + C_i W_i   (W_i = sum w_i)
  r_cross = a_3 Q + C_3 sum(ow_cross)
The P/Q pass, deep output matvec (row 6) and everything else accumulate in
one narrow PSUM group; a tiny PE transpose turns [8, 512] into per-row
scalars and the final combine runs on the otherwise-idle Pool engine.

Everything heavy runs in fp8e4m3 with DoubleRow perf mode (0.5 PE
cycles/row, two k-tiles per call): L0 (1024-padded x 1024), L1 (1024x512),
L2 (512x256) and the P/Q pass. Accuracy (gate 2e-2, measured 1.4e-2) is
held by residual compensation where it matters: Q = x . ow_cross feeds the
output directly, so it is computed as xhi.owh + (xhi.owl + xlo.owh)/16
(7-bit effective mantissas); P_i only enter via (1+P_i) ~ 1, so plain fp8
is fine. Host pre-quantizes x (*64, + *16 residual) and weights (*16); ACT
and DVE tensor_scalar ops fuse dequant+relu+requant via scale folding.

Schedule: software-pipelined across chunks with stage skew so the PE never
waits on ACT/DVE drain latency: iteration `it` runs L2+out for chunk it-2,
P/Q+L0 for chunk it, L1 for chunk it-1, the chunk it-3 tail (transpose +
Pool combine + store) interleaved, and chunk it+1's x DMA prefetched.
PSUM drains alternate ACT/DVE per tile to balance the two engines.
"""

import numpy as np
import ml_dtypes
from contextlib import ExitStack

import concourse.tile as tile
import concourse.mybir as mybir
from concourse import bacc
from concourse.bass_utils import run_bass_kernel_spmd

# ---- problem constants (hardcoded; kernel.py must be self-contained) ----
B, F, E = 16384, 26, 32
NF = 1_000_000
D = F * E                    # 832
DEEP = (1024, 512, 256)
N_CROSS = 3
N_CORES = 8
S = B // N_CORES             # 2048 batch rows per core
CHUNK = 512
NCHUNK = S // CHUNK          # 4
K8 = 8                       # fp8 k-tiles (1024 pad of 832)
KR = 2 * K8                  # hi + lo x tiles per chunk
D8 = K8 * 128                # 1024
M0, M1, M2 = DEEP[0] // 128, DEEP[1] // 128, DEEP[2] // 128  # 8, 4, 2

# fp8 scaling (powers of two; folded into ACT/DVE scales and Pool combine)
SX, SW0, SY0, SW1 = 64.0, 16.0, 64.0, 16.0
SY1, SW2 = 64.0, 16.0
SCW, SQC = 16.0, 16.0
SCL0 = SY0 / (SX * SW0)      # PSUM0 -> sy0*y0
SCL1 = SY1 / (SY0 * SW1)     # PSUM1 -> sy1*y1
SCL2 = 1.0 / (SY1 * SW2)     # PSUM2 -> y2 (natural)
C_Q = 1.0 / (SX * SQC)       # Q dequant (folded into the a-chain)
S_P = 1.0 / (SX * SCW)       # P dequant

NWARM = 8                    # PE p-state warm-up matmuls (tuned by sweep)
Y0_DVE = (1, 3, 5, 7)        # which y0 m-tiles drain on DVE (zb path)
Y1_DVE = (1, 3)
Y2_DVE = (0,)
SBQ_DVE = False
SPLIT_DMA = True
INLINE_L1 = False
EARLY_OUT = False
DPS_BUFS = 5
QPS_BUFS = 3
DRAIN_SEQ = (                # drain ordering (tuned by sweep)
    ("l2", 2), ("l1", 3), ("out", 2), ("tail", 1),
    ("l2", 3), ("out", 3), ("tail", 2), ("tail", 3),
)

_bf = mybir.dt.bfloat16
_f32 = mybir.dt.float32
_f8 = mybir.dt.float8e4
_np_bf = ml_dtypes.bfloat16
_np_f8 = ml_dtypes.float8_e4m3

_CACHE = {}


def _build_nc(zb=True, zk=True):
    """zb: all of b0/b1/b2 are zero -> half the relu drains use DVE
    2-op tensor_scalar forms; otherwise every drain runs on ACT with bias."""
    AF = mybir.ActivationFunctionType
    OP = mybir.AluOpType
    PM = mybir.MatmulPerfMode
    nc = bacc.Bacc(
        "TRN2", target_bir_lowering=False, debug=False, num_devices=N_CORES
    )

    x8_d = nc.dram_tensor("x8", [128, NCHUNK * KR * CHUNK], _f8, kind="ExternalInput")
    w0_d = nc.dram_tensor("w0", [128, K8 * DEEP[0]], _f8, kind="ExternalInput")
    w1_d = nc.dram_tensor("w1", [128, K8 * DEEP[1]], _f8, kind="ExternalInput")
    w2_d = nc.dram_tensor("w2", [128, M1 * DEEP[2]], _f8, kind="ExternalInput")
    pq8_d = nc.dram_tensor("pq8", [128, KR * 16], _f8, kind="ExternalInput")
    owd_d = nc.dram_tensor("owd", [128, M2 * 16], _bf, kind="ExternalInput")
    id_d = nc.dram_tensor("ident", [16, 16], _f32, kind="ExternalInput")
    # f32 consts: [b0*sy0 (8) | b1*sy1 (4) | b2 (2) | k1*cq k2*cq kf (3)]
    cst_d = nc.dram_tensor("cst", [128, M0 + M1 + M2 + 3], _f32, kind="ExternalInput")
    out_d = nc.dram_tensor("out", [S, 1], _f32, kind="ExternalOutput")

    with ExitStack() as ctx:
        tc = ctx.enter_context(tile.TileContext(nc))
        wp = ctx.enter_context(tc.tile_pool(name="wp", bufs=1))
        x8p = ctx.enter_context(tc.tile_pool(name="x8p", bufs=2))
        y0p = ctx.enter_context(tc.tile_pool(name="y0p", bufs=2))
        y1p = ctx.enter_context(tc.tile_pool(name="y1p", bufs=2))
        y2p = ctx.enter_context(tc.tile_pool(name="y2p", bufs=2))
        pqs = ctx.enter_context(tc.tile_pool(name="pqs", bufs=2))
        rp = ctx.enter_context(tc.tile_pool(name="rp", bufs=2))
        dps = ctx.enter_context(tc.tile_pool(name="dps", bufs=DPS_BUFS, space="PSUM"))
        qps = ctx.enter_context(tc.tile_pool(name="qps", bufs=QPS_BUFS, space="PSUM"))

        # ---- weights / constants to SBUF ----
        # DMA emission order == need order: x8_0 + w0 feed the first L0
        # groups (behind the warm-up burst); w1/w2/owd/ident load late.
        w0_sb = wp.tile([128, K8, DEEP[0]], _f8)
        w0_r = w0_d[:, :].rearrange("p (k m) -> p k m", k=K8)
        x8r = x8_d[:, :].rearrange("p (c k j) -> p c k j", c=NCHUNK, k=KR)
        x8t0 = x8p.tile([128, KR, CHUNK], _f8, tag="x8", name="x8_0")
        # k-tiles 7 (hi pad) and 15 (lo pad) are all-zero: memset on the idle
        # Pool engine instead of streaming zeros over DMA
        nc.gpsimd.memset(x8t0[:, K8 - 1, :], 0.0)
        nc.gpsimd.memset(x8t0[:, KR - 1, :], 0.0)
        nc.sync.dma_start(x8t0[:, 0:K8 - 1, :], x8r[:, 0, 0:K8 - 1, :])
        # w0 k-tile 7 (contraction rows 896..1023) is all-zero padding too
        nc.gpsimd.memset(w0_sb[:, K8 - 1, :], 0.0)
        nc.sync.dma_start(w0_sb[:, 0:K8 - 1, 0:512], w0_r[:, 0:K8 - 1, 0:512])
        nc.sync.dma_start(x8t0[:, K8:KR - 1, :], x8r[:, 0, K8:KR - 1, :])
        pq8_sb = wp.tile([128, KR, 16], _f8)
        nc.sync.dma_start(pq8_sb[:], pq8_d[:, :].rearrange("p (k c) -> p k c", k=KR))
        cst_sb = wp.tile([128, M0 + M1 + M2 + 3], _f32)
        nc.sync.dma_start(cst_sb[:], cst_d[:, :])
        b0_sb = cst_sb[:, 0:M0]
        b1_sb = cst_sb[:, M0:M0 + M1]
        b2_sb = cst_sb[:, M0 + M1:M0 + M1 + M2]
        kv_sb = cst_sb[:, M0 + M1 + M2:M0 + M1 + M2 + 3]
        nc.sync.dma_start(w0_sb[:, 0:K8 - 1, 512:1024], w0_r[:, 0:K8 - 1, 512:1024])
        id_sb = wp.tile([16, 16], _f32)
        w1_sb = wp.tile([128, K8, DEEP[1]], _f8)
        w2_sb = wp.tile([128, M1, DEEP[2]], _f8)
        owd_sb = wp.tile([128, M2, 16], _bf)

        def _late_loads():
            nc.sync.dma_start(w1_sb[:], w1_d[:, :].rearrange("p (k m) -> p k m", k=K8))
            nc.sync.dma_start(w2_sb[:], w2_d[:, :].rearrange("p (k m) -> p k m", k=M1))
            nc.sync.dma_start(owd_sb[:], owd_d[:, :].rearrange("p (k c) -> p k c", k=M2))
            nc.sync.dma_start(id_sb[:], id_d[:, :])

        # "Observe" ops: each engine touches its DMA-loaded constants once so
        # steady-state instructions carry at most one semaphore wait.
        obs = wp.tile([128, 8], _f32)
        nc.vector.tensor_copy(obs[:, 0:1], kv_sb[:, 0:1])
        nc.gpsimd.tensor_copy(obs[:, 4:5], kv_sb[:, 0:1])
        nc.scalar.activation(obs[:, 1:2], b0_sb[:, 0:1], AF.Copy)
        nc.scalar.activation(obs[:, 2:3], b1_sb[:, 0:1], AF.Copy)
        nc.scalar.activation(obs[:, 3:4], b2_sb[:, 0:1], AF.Copy)
        # PE warm-up burst: keep the PE busy during the startup DMA window so
        # the clock p-state ramps before the first real matmul group.
        warm = wp.tile([128, 512], _bf)
        nc.gpsimd.memset(warm[:], 0.0)
        warm_ps = dps.tile([128, 512], _f32, tag="dps", name="warm_ps")
        for _ in range(NWARM):
            nc.tensor.matmul(
                warm_ps[:], lhsT=warm[:, 0:128], rhs=warm[:], start=True, stop=True
            )
        for w_ap in (pq8_sb[:, 0, 0:1], w0_sb[:, 0, 0:1]):
            nc.tensor.matmul(
                warm_ps[0:1, 0:1], lhsT=w_ap, rhs=w_ap, start=True, stop=True
            )

        # ---- per-chunk stage emitters ----
        x8ts = {0: x8t0}
        y0ts, y1ts, y2ts, qts, sbqs = {}, {}, {}, {}, {}

        def dma_x(c):
            x8t = x8p.tile([128, KR, CHUNK], _f8, tag="x8", name=f"x8_{c}")
            nc.gpsimd.memset(x8t[:, K8 - 1, :], 0.0)
            nc.gpsimd.memset(x8t[:, KR - 1, :], 0.0)
            nc.sync.dma_start(x8t[:, 0:K8 - 1, :], x8r[:, c, 0:K8 - 1, :])
            nc.sync.dma_start(x8t[:, K8:KR - 1, :], x8r[:, c, K8:KR - 1, :])
            x8ts[c] = x8t

        def pq_open(c):
            # P/Q DoubleRow group over hi tiles (P, Q1, Q2) and lo tiles (Q3)
            qt = qps.tile([16, CHUNK], _f32, tag="pq", name=f"pq_{c}")
            qts[c] = qt
            for kp in range(KR // 2):
                nc.tensor.matmul(
                    qt[:],
                    lhsT=pq8_sb[:, 2 * kp:2 * kp + 2, :],
                    rhs=x8ts[c][:, 2 * kp:2 * kp + 2, :],
                    start=(kp == 0),
                    stop=False,
                    perf_mode=PM.DoubleRow,
                    skip_group_check=True,
                )

        def l0_alloc(c):
            y0ts[c] = y0p.tile([128, K8, CHUNK], _f8, tag="y0", name=f"y0_{c}")

        def l0(c, m):
            ps = dps.tile([128, CHUNK], _f32, tag="dps", name=f"ps0_{c}_{m}")
            for kp in range(K8 // 2):
                nc.tensor.matmul(
                    ps[:],
                    lhsT=w0_sb[:, 2 * kp:2 * kp + 2, m * 128:(m + 1) * 128],
                    rhs=x8ts[c][:, 2 * kp:2 * kp + 2, :],
                    start=(kp == 0),
                    stop=(kp == K8 // 2 - 1),
                    perf_mode=PM.DoubleRow,
                )
            # drain-balance: even m on ACT, odd m on DVE (zero-bias form)
            if zb and m in Y0_DVE:
                nc.vector.tensor_scalar(
                    y0ts[c][:, m, :], ps[:], SCL0, 0.0, op0=OP.mult, op1=OP.max
                )
            else:
                nc.scalar.activation(
                    y0ts[c][:, m, :], ps[:], AF.Relu, bias=b0_sb[:, m:m + 1], scale=SCL0
                )

        def l1(c):
            y1t = y1p.tile([128, M1, CHUNK], _f8, tag="y1", name=f"y1_{c}")
            y1ts[c] = y1t
            for m in range(M1):
                ps = dps.tile([128, CHUNK], _f32, tag="dps", name=f"ps1_{c}_{m}")
                for kp in range(K8 // 2):
                    nc.tensor.matmul(
                        ps[:],
                        lhsT=w1_sb[:, 2 * kp:2 * kp + 2, m * 128:(m + 1) * 128],
                        rhs=y0ts[c][:, 2 * kp:2 * kp + 2, :],
                        start=(kp == 0),
                        stop=(kp == K8 // 2 - 1),
                        perf_mode=PM.DoubleRow,
                    )
                if zb and m in Y1_DVE:
                    nc.vector.tensor_scalar(
                        y1t[:, m, :], ps[:], SCL1, 0.0, op0=OP.mult, op1=OP.max
                    )
                else:
                    nc.scalar.activation(
                        y1t[:, m, :], ps[:], AF.Relu, bias=b1_sb[:, m:m + 1], scale=SCL1
                    )

        def l2(c):
            y2t = y2p.tile([128, M2, CHUNK], _bf, tag="y2", name=f"y2_{c}")
            y2ts[c] = y2t
            for m in range(M2):
                ps = dps.tile([128, CHUNK], _f32, tag="dps", name=f"ps2_{c}_{m}")
                for kp in range(M1 // 2):
                    nc.tensor.matmul(
                        ps[:],
                        lhsT=w2_sb[:, 2 * kp:2 * kp + 2, m * 128:(m + 1) * 128],
                        rhs=y1ts[c][:, 2 * kp:2 * kp + 2, :],
                        start=(kp == 0),
                        stop=(kp == M1 // 2 - 1),
                        perf_mode=PM.DoubleRow,
                    )
                if zb and m in Y2_DVE:
                    nc.vector.tensor_scalar(
                        y2t[:, m, :], ps[:], SCL2, 0.0, op0=OP.mult, op1=OP.max
                    )
                else:
                    nc.scalar.activation(
                        y2t[:, m, :], ps[:], AF.Relu, bias=b2_sb[:, m:m + 1],
                        scale=SCL2,
                    )

        def out_mv(c, j):
            # deep output matvec accumulates into row 6 of the P/Q group
            nc.tensor.matmul(
                qts[c][:],
                lhsT=owd_sb[:, j, :],
                rhs=y2ts[c][:, j, :],
                start=False,
                stop=(j == M2 - 1),
                skip_group_check=True,
            )
            if j == M2 - 1:
                sbq = pqs.tile([16, CHUNK], _f32, tag="sbq", name=f"sbq_{c}")
                if SBQ_DVE:
                    nc.vector.tensor_copy(sbq[:], qts[c][:])
                else:
                    nc.scalar.activation(sbq[:], qts[c][:], AF.Copy)
                sbqs[c] = sbq

        def tail(c):
            sbq = sbqs[c]
            # transpose scratch borrows a [128,512] slot from the dps ring
            pt = dps.tile([128, CHUNK], _f32, tag="dps", name=f"ptr_{c}")
            ptr = pt[:, 0:64].rearrange("p (s i) -> p s i", s=4)
            for s in range(4):
                nc.tensor.transpose(
                    ptr[:, s, :], sbq[:, s * 128:(s + 1) * 128], id_sb[:]
                )
            # one small ACT copy frees the PSUM slot; the scalar combine then
            # runs entirely on the otherwise-idle Pool engine from SBUF.
            # pb rows: 0-2 P_i, 3 Q1 (hi.hi), 4 Q2 (hi.lo), 5 Q3 (lo.hi),
            # 6 Rdeep.  The a-chain carries the C_Q fold: t1 = (1+P0)*C_Q.
            pb = rp.tile([128, 4, 16], _f32, tag="pb", name=f"pb_{c}")
            nc.scalar.activation(pb[:], ptr[:], AF.Copy)
            t1 = rp.tile([128, 4], _f32, tag="t1", name=f"t1_{c}")
            t2 = rp.tile([128, 4], _f32, tag="t2", name=f"t2_{c}")
            t3 = rp.tile([128, 4], _f32, tag="t3", name=f"t3_{c}")
            nc.gpsimd.tensor_scalar(
                t1[:], pb[:, :, 0], S_P * C_Q, C_Q, op0=OP.mult, op1=OP.add
            )
            nc.gpsimd.tensor_scalar(
                t2[:], pb[:, :, 1], S_P, 1.0, op0=OP.mult, op1=OP.add
            )
            nc.gpsimd.tensor_scalar(
                t3[:], pb[:, :, 2], S_P, 1.0, op0=OP.mult, op1=OP.add
            )
            u = rp.tile([128, 4], _f32, tag="u", name=f"u_{c}")
            nc.gpsimd.tensor_tensor(out=u[:], in0=pb[:, :, 4], in1=pb[:, :, 5], op=OP.add)
            qv = rp.tile([128, 4], _f32, tag="qv", name=f"qv_{c}")
            nc.gpsimd.tensor_scalar(qv[:], u[:], 1.0 / 16.0, None, op0=OP.mult)
            nc.gpsimd.tensor_tensor(out=qv[:], in0=qv[:], in1=pb[:, :, 3], op=OP.add)
            acc = rp.tile([128, 4], _f32, tag="acc", name=f"acc_{c}")
            nc.gpsimd.tensor_tensor(out=acc[:], in0=t1[:], in1=t2[:], op=OP.mult)
            if not zb:
                nc.gpsimd.tensor_scalar_add(acc[:], acc[:], kv_sb[:, 0:1])
            nc.gpsimd.tensor_tensor(out=acc[:], in0=acc[:], in1=t3[:], op=OP.mult)
            if not zb:
                nc.gpsimd.tensor_scalar_add(acc[:], acc[:], kv_sb[:, 1:2])
            nc.gpsimd.tensor_tensor(out=acc[:], in0=acc[:], in1=qv[:], op=OP.mult)
            res = rp.tile([128, 4], _f32, tag="res", name=f"res_{c}")
            if zk:
                nc.gpsimd.tensor_tensor(out=res[:], in0=acc[:], in1=pb[:, :, 6], op=OP.add)
            else:
                nc.gpsimd.tensor_tensor(out=acc[:], in0=acc[:], in1=pb[:, :, 6], op=OP.add)
                nc.gpsimd.tensor_scalar_add(res[:], acc[:], kv_sb[:, 2:3])
            nc.sync.dma_start(
                out=out_d[c * CHUNK:(c + 1) * CHUNK, :].rearrange(
                    "(s p) o -> p (s o)", p=128
                ),
                in_=res[:],
            )

        # ---- software-pipelined main loop ----
        for it in range(NCHUNK):
            A, Bc, Cc, Tc = it, it - 1, it - 2, it - 3
            if A == 0:
                l0_alloc(0)
                for m in range(4):
                    l0(0, m)
                pq_open(0)
                for m in range(4, M0):
                    l0(0, m)
                dma_x(1)
                _late_loads()
                continue
            if 0 <= Cc:
                l2(Cc)
            if A + 1 < NCHUNK:
                dma_x(A + 1)
            l0_alloc(A)
            l0(A, 0)
            if 0 <= Tc:
                tail(Tc)
            l0(A, 1)
            if EARLY_OUT and 0 <= Cc:
                out_mv(Cc, 0)
                out_mv(Cc, 1)
            pq_open(A)
            l0(A, 2)
            l0(A, 3)
            if 0 <= Bc:
                l1(Bc)
            for m in range(4, M0):
                l0(A, m)
            # last chunk: L1 inline (PE is idle in the drain anyway)
            if INLINE_L1 and A == NCHUNK - 1:
                l1(A)
            if not EARLY_OUT and 0 <= Cc:
                out_mv(Cc, 0)
                out_mv(Cc, 1)
        # ---- compressed drain: PE is idle, so collapse the stage skew ----
        for step in DRAIN_SEQ:
            kind, c = step
            if kind == "l2":
                l2(c)
            elif kind == "out":
                out_mv(c, 0)
                out_mv(c, 1)
            elif kind == "l1":
                l1(c)
            elif kind == "tail":
                tail(c)

    nc.compile()
    return nc


def _get_nc(zb=True, zk=True):
    key = f"nc_zb{int(zb)}_zk{int(zk)}"
    if key not in _CACHE:
        _CACHE[key] = _build_nc(zb=zb, zk=zk)
    return _CACHE[key]


def _prep_in_maps(inputs, zb):
    fi = np.asarray(inputs["feature_index"]).astype(np.int64)
    fvv = np.asarray(inputs["feature_value"], dtype=np.float32)
    emb = np.asarray(inputs["emb_table"], dtype=np.float32)
    cw = np.asarray(inputs["cross_w"], dtype=np.float32)
    cb = np.asarray(inputs["cross_b"], dtype=np.float32)
    w0 = np.asarray(inputs["w0"], dtype=np.float32)
    b0 = np.asarray(inputs["b0"], dtype=np.float32)
    w1 = np.asarray(inputs["w1"], dtype=np.float32)
    b1 = np.asarray(inputs["b1"], dtype=np.float32)
    w2 = np.asarray(inputs["w2"], dtype=np.float32)
    b2 = np.asarray(inputs["b2"], dtype=np.float32)
    ow = np.asarray(inputs["out_w"], dtype=np.float32).reshape(-1)
    ob = np.asarray(inputs["out_b"], dtype=np.float32).reshape(-1)

    # host gather with feature_value premultiplied (f32, before any cast)
    x = emb[fi] * fvv[:, :, None]               # [B, F, E] f32
    x = x.reshape(B, D)

    # hi/lo fp8 split of x (lo = 16x the hi-quantization residual)
    xs = np.zeros((B, D8), dtype=np.float32)
    xs[:, :D] = x * SX
    xhi = xs.astype(_np_f8)
    xlo = ((xs - xhi.astype(np.float32)) * 16.0).astype(_np_f8)

    # shared (replicated) weights
    w0p = np.zeros((D8, DEEP[0]), dtype=np.float32)
    w0p[:D] = w0 * SW0
    w0q = np.ascontiguousarray(
        w0p.reshape(K8, 128, DEEP[0]).transpose(1, 0, 2).reshape(128, -1)
    ).astype(_np_f8)
    w1q = np.ascontiguousarray(
        (w1 * SW1).reshape(K8, 128, DEEP[1]).transpose(1, 0, 2).reshape(128, -1)
    ).astype(_np_f8)
    w2q = np.ascontiguousarray(
        (w2 * SW2).reshape(M1, 128, DEEP[2]).transpose(1, 0, 2).reshape(128, -1)
    ).astype(_np_f8)

    # P/Q lhsT: hi k-tiles get [cw0 cw1 cw2 owch owcl 0 0 0]; lo k-tiles get
    # owch in column 5 (-> Q3).  All quantized fp8.
    owc = np.zeros((D8,), dtype=np.float32)
    owc[:D] = ow[:D]
    owch = (owc * SQC).astype(_np_f8)
    owcl = ((owc * SQC - owch.astype(np.float32)) * 16.0).astype(_np_f8)
    cwp = np.zeros((N_CROSS, D8), dtype=np.float32)
    cwp[:, :D] = cw * SCW
    pq8 = np.zeros((128, KR, 16), dtype=_np_f8)
    for k in range(K8):
        seg = slice(k * 128, (k + 1) * 128)
        for i in range(N_CROSS):
            pq8[:, k, i] = cwp[i, seg].astype(_np_f8)
        pq8[:, k, 3] = owch[seg]
        pq8[:, k, 4] = owcl[seg]
        pq8[:, K8 + k, 5] = owch[seg]
    pq8 = np.ascontiguousarray(pq8.reshape(128, -1))
    owd = np.zeros((128, M2, 16), dtype=np.float32)
    for j in range(M2):
        owd[:, j, 6] = ow[D + j * 128:D + (j + 1) * 128]
    owd = np.ascontiguousarray(owd.reshape(128, -1)).astype(_np_bf)

    C = np.cumsum(cb)                           # C[i] = cb_0 + ... + cb_i
    W = cw.sum(axis=1)
    k1 = C[0] * W[1] * C_Q
    k2 = C[1] * W[2] * C_Q
    kf = ob[0] + C[2] * ow[:D].sum()
    b0s = (b0 * SY0).reshape(M0, 128).T
    b1s = (b1 * SY1).reshape(M1, 128).T
    b2r = b2.reshape(M2, 128).T
    kv = np.tile(np.array([[k1, k2, kf]], dtype=np.float32), (128, 1))
    cst = np.ascontiguousarray(
        np.concatenate([b0s, b1s, b2r, kv], axis=1).astype(np.float32)
    )
    ident = np.eye(16, dtype=np.float32)

    shared = dict(w0=w0q, w1=w1q, w2=w2q, pq8=pq8, owd=owd, cst=cst, ident=ident)

    in_maps = []
    for core in range(N_CORES):
        rows = slice(core * S, (core + 1) * S)
        # per-chunk layout [128, c, k(hi 0-7, lo 8-15), j]
        xh = xhi[rows].reshape(NCHUNK, CHUNK, K8, 128).transpose(3, 0, 2, 1)
        xl = xlo[rows].reshape(NCHUNK, CHUNK, K8, 128).transpose(3, 0, 2, 1)
        x8 = np.concatenate([xh, xl], axis=2).reshape(128, -1)
        in_maps.append(dict(x8=np.ascontiguousarray(x8), **shared))
    return in_maps


def _zb(inputs):
    return not (
        np.any(np.asarray(inputs["b0"])) or np.any(np.asarray(inputs["b1"]))
        or np.any(np.asarray(inputs["b2"]))
    )


def _kf(inputs):
    cb = np.asarray(inputs["cross_b"], dtype=np.float32)
    ow = np.asarray(inputs["out_w"], dtype=np.float32).reshape(-1)
    ob = np.asarray(inputs["out_b"], dtype=np.float32).reshape(-1)
    return float(ob[0] + np.cumsum(cb)[2] * ow[:D].sum())


def _run(inputs, trace=False, **kw):
    zb = _zb(inputs)
    nc = _get_nc(zb=zb, zk=(_kf(inputs) == 0.0))
    in_maps = _prep_in_maps(inputs, zb)
    res = run_bass_kernel_spmd(
        nc, in_maps, core_ids=list(range(N_CORES)), trace=trace, **kw
    )
    out = np.concatenate([r["out"] for r in res.results], axis=0)
    return out.astype(np.float32), res


def kernel(**inputs) -> np.ndarray:
    out, _ = _run(inputs, trace=False)
    return out
